# revision 3
# baseline (speedup 1.0000x reference)
"""Two-layer GCN (PyG GCNConv x2 + ReLU) on 8 Trainium2 NeuronCores.

Math: out = relu(S @ (relu(S @ (x W1) + b1) W2) + b2), with
S = D^-1/2 (A + I) D^-1/2 the symmetric-normalized adjacency (1.6M random
edges + self loops over 50000 nodes).

Key reformulation: aggregation is linear, so S (x W) == (S x) W.  Both
layers aggregate in the *small* (64-wide) feature space:
  layer1: agg = S x            (64 wide), h1 = relu(agg @ W1 + b1)
  layer2: agg = S (h1 W2)?  -> actually h1 is 128 wide; we aggregate h1
          directly (128 wide) and transform after: relu((S h1) @ W2 + b2).

Sharding: destination nodes are range-partitioned across the 8 cores
(6250 per core).  Each core owns the edges whose dst falls in its range
(plus its self-loops), pre-sorted by dst tile on the host.  Layer-1's
gather table is the (replicated) input x itself; layer-2's table is the
AllGather of the per-core h1 shards.  Weights are replicated.

Per 128-edge chunk the device does:
  - dma_gather: 128 rows of the node table -> SBUF [128 edges, 128 feats]
  - DVE tensor_scalar builds V[e,d] = (iota[d]==dstl[e]) * norm[e]
    (norm = dis[src]*dis[dst] precomputed per edge on the host; self
    loops are ordinary edges with norm = dis^2)
  - PE matmul accumulates agg.T[f,d] += gathered.T @ V in PSUM
Per 128-dst tile: transform with W (PE), bias+relu (ACT, bias is
per-partition in the transposed layout), transpose back to node-major
(PE), and DMA out.
"""

import math

import numpy as np

# ---------------------------------------------------------------------------
# Configuration
# ---------------------------------------------------------------------------


class Cfg:
    def __init__(self, n, n_cores, half, batch_tiles, c0, c1):
        self.n = n                       # total nodes
        self.n_cores = n_cores
        self.npc = n // n_cores          # nodes (dsts) per core
        self.nt = (self.npc + 127) // 128  # dst tiles per core
        self.last_rows = self.npc - (self.nt - 1) * 128
        self.half = half                 # table split point (int16 idx range)
        self.c0 = c0                     # chunks per tile, src < half
        self.c1 = c1                     # chunks per tile, src >= half
        self.nchunk = self.nt * (c0 + c1)
        self.batches = []
        t = 0
        while t < self.nt:
            self.batches.append(list(range(t, min(t + batch_tiles, self.nt))))
            t += batch_tiles
        self.f0, self.f1, self.f2 = 64, 128, 64


# ---------------------------------------------------------------------------
# Host-side preprocessing (graph partitioning / normalization structure)
# ---------------------------------------------------------------------------


def prepare(x, edge_index, W1, b1, W2, b2, n_cores=8, half=32768, batch_tiles=2):
    x = np.asarray(x, dtype=np.float32)
    edge_index = np.asarray(edge_index)
    W1 = np.asarray(W1, dtype=np.float32)
    b1 = np.asarray(b1, dtype=np.float32)
    W2 = np.asarray(W2, dtype=np.float32)
    b2 = np.asarray(b2, dtype=np.float32)

    n = x.shape[0]
    src = edge_index[0].astype(np.int64)
    dst = edge_index[1].astype(np.int64)

    deg = 1.0 + np.bincount(dst, minlength=n).astype(np.float64)
    dis = (1.0 / np.sqrt(deg)).astype(np.float32)

    # self loops as ordinary edges
    loops = np.arange(n, dtype=np.int64)
    src_all = np.concatenate([src, loops])
    dst_all = np.concatenate([dst, loops])
    norm_all = dis[src_all] * dis[dst_all]

    npc = n // n_cores
    core_of = dst_all // npc

    # pass 1: per-(core, tile, half) counts -> global uniform chunk counts
    per_core = []
    c0_max = 1
    c1_max = 1
    nt = (npc + 127) // 128
    for c in range(n_cores):
        m = core_of == c
        s = src_all[m]
        d = dst_all[m] - c * npc
        nr = norm_all[m]
        tile_id = d >> 7
        h = (s >= half).astype(np.int64)
        key = tile_id * 2 + h
        order = np.argsort(key, kind="stable")
        sk = key[order]
        bounds = np.searchsorted(sk, np.arange(nt * 2 + 1))
        cnt = np.diff(bounds)
        c0_max = max(c0_max, int(math.ceil(cnt[0::2].max() / 128.0)))
        c1_max = max(c1_max, int(math.ceil(cnt[1::2].max() / 128.0)))
        per_core.append((s, d, nr, order, bounds))

    cfg = Cfg(n, n_cores, half, batch_tiles, c0_max, c1_max)

    # pass 2: build padded streams in batch order
    maps = []
    for c in range(n_cores):
        s, d, nr, order, bounds = per_core[c]
        idx_stream = np.zeros(cfg.nchunk * 128, np.int16)
        dstl_stream = np.full(cfg.nchunk * 128, 500.0, np.float32)
        norm_stream = np.zeros(cfg.nchunk * 128, np.float32)
        pos = 0  # in chunks
        for batch in cfg.batches:
            for hh, cmax in ((0, cfg.c0), (1, cfg.c1)):
                for t in batch:
                    seg = order[bounds[2 * t + hh]:bounds[2 * t + hh + 1]]
                    L = len(seg)
                    base = pos * 128
                    sv = s[seg]
                    if hh:
                        sv = sv - half
                    idx_stream[base:base + L] = sv.astype(np.int16)
                    dstl_stream[base:base + L] = (d[seg] & 127).astype(np.float32)
                    norm_stream[base:base + L] = nr[seg]
                    pos += cmax
        assert pos == cfg.nchunk
        eidx = np.tile(idx_stream.reshape(cfg.nchunk * 8, 16).T, (8, 1))
        edstl = np.ascontiguousarray(dstl_stream.reshape(cfg.nchunk, 128).T)
        enorm = np.ascontiguousarray(norm_stream.reshape(cfg.nchunk, 128).T)
        maps.append({"eidx": eidx, "edstl": edstl, "enorm": enorm})

    xt = np.zeros((n, 128), np.float16)
    xt[:, :64] = x
    shared = {
        "xt": xt,
        "w1": W1.astype(np.float16),
        "b1": np.ascontiguousarray(b1.reshape(cfg.f1, 1)),
        "w2": W2.astype(np.float16),
        "b2": np.ascontiguousarray(b2.reshape(cfg.f2, 1)),
        "iota": np.tile(np.arange(128, dtype=np.float16), (128, 1)),
        "idA": np.eye(128, dtype=np.float16),
        "idB": np.eye(cfg.f2, dtype=np.float32),
    }
    in_maps = [{**shared, **m} for m in maps]
    return cfg, in_maps


# ---------------------------------------------------------------------------
# Device program
# ---------------------------------------------------------------------------


def build_program(cfg):
    import concourse.bacc as bacc
    import concourse.mybir as mybir
    import concourse.tile as tile

    dt = mybir.dt
    AF = mybir.ActivationFunctionType
    ALU = mybir.AluOpType

    n, npc, nt, half = cfg.n, cfg.npc, cfg.nt, cfg.half
    c0, c1 = cfg.c0, cfg.c1
    nchunk = cfg.nchunk
    F0, F1, F2 = cfg.f0, cfg.f1, cfg.f2

    nc = bacc.Bacc(
        "TRN2",
        target_bir_lowering=False,
        debug=False,
        enable_asserts=False,
        num_devices=cfg.n_cores,
    )

    xt = nc.dram_tensor("xt", [n, 128], dt.float16, kind="ExternalInput")
    eidx = nc.dram_tensor("eidx", [128, nchunk * 8], dt.int16, kind="ExternalInput")
    edstl = nc.dram_tensor("edstl", [128, nchunk], dt.float32, kind="ExternalInput")
    enorm = nc.dram_tensor("enorm", [128, nchunk], dt.float32, kind="ExternalInput")
    w1t = nc.dram_tensor("w1", [F0, F1], dt.float16, kind="ExternalInput")
    b1t = nc.dram_tensor("b1", [F1, 1], dt.float32, kind="ExternalInput")
    w2t = nc.dram_tensor("w2", [F1, F2], dt.float16, kind="ExternalInput")
    b2t = nc.dram_tensor("b2", [F2, 1], dt.float32, kind="ExternalInput")
    iotat = nc.dram_tensor("iota", [128, 128], dt.float16, kind="ExternalInput")
    idAt = nc.dram_tensor("idA", [128, 128], dt.float16, kind="ExternalInput")
    idBt = nc.dram_tensor("idB", [F2, F2], dt.float32, kind="ExternalInput")
    outt = nc.dram_tensor("out", [npc, F2], dt.float32, kind="ExternalOutput")

    with tile.TileContext(nc) as tc:
        with (
            tc.tile_pool(name="const", bufs=1) as cp,
            tc.tile_pool(name="edges", bufs=1) as ep,
            tc.tile_pool(name="gb", bufs=2) as gp,
            tc.tile_pool(name="v", bufs=8) as vp,
            tc.tile_pool(name="sb", bufs=3) as sp,
            tc.tile_pool(name="psA", bufs=2, space="PSUM") as psA,
            tc.tile_pool(name="psZ", bufs=2, space="PSUM") as psZ,
            tc.tile_pool(name="psT", bufs=2, space="PSUM") as psT,
            tc.tile_pool(name="dram", bufs=1, space="DRAM") as dp,
        ):
            def load_const(pool, t, dtype):
                sb = pool.tile(list(t.shape), dtype, tag=t.name)
                nc.sync.dma_start(sb[:], t.ap()[:])
                return sb

            iota_sb = load_const(cp, iotat, dt.float16)
            idA_sb = load_const(cp, idAt, dt.float16)
            idB_sb = load_const(cp, idBt, dt.float32)
            w1_sb = load_const(cp, w1t, dt.float16)
            b1_sb = load_const(cp, b1t, dt.float32)
            w2_sb = load_const(cp, w2t, dt.float16)
            b2_sb = load_const(cp, b2t, dt.float32)
            eidx_sb = load_const(ep, eidx, dt.int16)
            edstl_sb = load_const(ep, edstl, dt.float32)
            enorm_sb = load_const(ep, enorm, dt.float32)

            cc_in = dp.tile([npc, F1], dt.float16)
            cc_out = dp.tile([n, F1], dt.float16)

            n_regs = {}
            for bt in {len(b) for b in cfg.batches}:
                n_regs[bt * c0 * 128] = nc.gpsimd.to_reg(bt * c0 * 128)
                n_regs[bt * c1 * 128] = nc.gpsimd.to_reg(bt * c1 * 128)

            def do_layer(table, f_in, w_sb, b_sb, out_dt, id_sb, f_out, write_out):
                g_base = 0
                for batch in cfg.batches:
                    bt = len(batch)
                    nch = bt * (c0 + c1)
                    gb = gp.tile([128, nch, 128], dt.float16, tag="gb")
                    n0 = bt * c0 * 128
                    n1 = bt * c1 * 128
                    nc.gpsimd.dma_gather(
                        gb[:, 0:bt * c0, :],
                        table[0:half, :],
                        eidx_sb[:, g_base * 8:(g_base + bt * c0) * 8],
                        num_idxs=n0,
                        num_idxs_reg=n_regs[n0],
                        elem_size=128,
                        single_packet=False,
                    )
                    nc.gpsimd.dma_gather(
                        gb[:, bt * c0:nch, :],
                        table[half:n, :],
                        eidx_sb[:, (g_base + bt * c0) * 8:(g_base + nch) * 8],
                        num_idxs=n1,
                        num_idxs_reg=n_regs[n1],
                        elem_size=128,
                        single_packet=False,
                    )
                    for i, t in enumerate(batch):
                        agg_ps = psA.tile([f_in, 128], dt.float32, tag="psA")
                        slots = list(range(i * c0, (i + 1) * c0)) + list(
                            range(bt * c0 + i * c1, bt * c0 + (i + 1) * c1)
                        )
                        for j, s in enumerate(slots):
                            g = g_base + s
                            V = vp.tile([128, 128], dt.float16, tag="v")
                            nc.vector.tensor_scalar(
                                V[:],
                                iota_sb[:],
                                edstl_sb[:, g:g + 1],
                                enorm_sb[:, g:g + 1],
                                ALU.is_equal,
                                ALU.mult,
                            )
                            nc.tensor.matmul(
                                agg_ps[:],
                                gb[:, s, 0:f_in],
                                V[:],
                                start=(j == 0),
                                stop=(j == len(slots) - 1),
                            )
                        agg_sb = sp.tile([f_in, 128], dt.float16, tag="agg")
                        nc.vector.tensor_copy(agg_sb[:], agg_ps[:])
                        z_ps = psZ.tile([f_out, 128], dt.float32, tag="psZ")
                        nc.tensor.matmul(z_ps[:], w_sb[:], agg_sb[:], start=True, stop=True)
                        zr_sb = sp.tile([f_out, 128], out_dt, tag="zr")
                        nc.scalar.activation(zr_sb[:], z_ps[:], AF.Relu, bias=b_sb[:], scale=1.0)
                        tr_ps = psT.tile([128, f_out], out_dt, tag="psT")
                        nc.tensor.transpose(tr_ps[:], zr_sb[:], id_sb[:])
                        h_sb = sp.tile([128, f_out], out_dt, tag="h")
                        nc.vector.tensor_copy(h_sb[:], tr_ps[:])
                        rows = cfg.last_rows if t == nt - 1 else 128
                        write_out(t, h_sb, rows)
                    g_base += nch

            def w1_out(t, h_sb, rows):
                nc.sync.dma_start(cc_in[t * 128:t * 128 + rows, :], h_sb[0:rows, :])

            do_layer(xt.ap(), F0, w1_sb, b1_sb, dt.float16, idA_sb, F1, w1_out)

            nc.gpsimd.collective_compute(
                "AllGather",
                ALU.bypass,
                replica_groups=[list(range(cfg.n_cores))],
                ins=[cc_in.opt()],
                outs=[cc_out.opt()],
            )

            def w2_out(t, h_sb, rows):
                nc.sync.dma_start(outt.ap()[t * 128:t * 128 + rows, :], h_sb[0:rows, :])

            do_layer(cc_out[:, :], F1, w2_sb, b2_sb, dt.float32, idB_sb, F2, w2_out)

    nc.compile()
    return nc


# ---------------------------------------------------------------------------
# Entry point
# ---------------------------------------------------------------------------

_CACHE = {}


def kernel(x, edge_index, W1, b1, W2, b2):
    x = np.asarray(x)
    cfg, in_maps = prepare(x, edge_index, W1, b1, W2, b2)

    key = (cfg.n, cfg.n_cores, cfg.c0, cfg.c1)
    nc = _CACHE.get(key)
    if nc is None:
        nc = build_program(cfg)
        _CACHE[key] = nc

    from concourse.bass_utils import run_bass_kernel_spmd

    res = run_bass_kernel_spmd(nc, in_maps, core_ids=list(range(cfg.n_cores)))
    out = np.concatenate([r["out"] for r in res.results], axis=0)
    return np.ascontiguousarray(out.astype(np.float32))


# revision 4
# speedup vs baseline: 1.3906x; 1.3906x over previous
"""Two-layer GCN (PyG GCNConv x2 + ReLU) on 8 Trainium2 NeuronCores.

Math: out = relu(S @ (relu(S @ (x W1) + b1) W2) + b2), with
S = D^-1/2 (A + I) D^-1/2 the symmetric-normalized adjacency (1.6M random
edges + self loops over 50000 nodes).

Key reformulation: aggregation is linear, so S (x W) == (S x) W.  Both
layers aggregate in the *small* (64-wide) feature space:
  layer1: agg = S x            (64 wide), h1 = relu(agg @ W1 + b1)
  layer2: agg = S (h1 W2)?  -> actually h1 is 128 wide; we aggregate h1
          directly (128 wide) and transform after: relu((S h1) @ W2 + b2).

Sharding: destination nodes are range-partitioned across the 8 cores
(6250 per core).  Each core owns the edges whose dst falls in its range
(plus its self-loops), pre-sorted by dst tile on the host.  Layer-1's
gather table is the (replicated) input x itself; layer-2's table is the
AllGather of the per-core h1 shards.  Weights are replicated.

Per 128-edge chunk the device does:
  - dma_gather: 128 rows of the node table -> SBUF [128 edges, 128 feats]
  - DVE tensor_scalar builds V[e,d] = (iota[d]==dstl[e]) * norm[e]
    (norm = dis[src]*dis[dst] precomputed per edge on the host; self
    loops are ordinary edges with norm = dis^2)
  - PE matmul accumulates agg.T[f,d] += gathered.T @ V in PSUM
Per 128-dst tile: transform with W (PE), bias+relu (ACT, bias is
per-partition in the transposed layout), transpose back to node-major
(PE), and DMA out.
"""

import math

import numpy as np

# ---------------------------------------------------------------------------
# Configuration
# ---------------------------------------------------------------------------


class Cfg:
    def __init__(self, n, n_cores, half, batch_tiles, c0, c1):
        self.n = n                       # total nodes
        self.n_cores = n_cores
        self.npc = n // n_cores          # nodes (dsts) per core
        self.nt = (self.npc + 127) // 128  # dst tiles per core
        self.last_rows = self.npc - (self.nt - 1) * 128
        self.half = half                 # table split point (int16 idx range)
        self.c0 = c0                     # chunks per tile, src < half
        self.c1 = c1                     # chunks per tile, src >= half
        self.nchunk = self.nt * (c0 + c1)
        self.batches = []
        t = 0
        while t < self.nt:
            self.batches.append(list(range(t, min(t + batch_tiles, self.nt))))
            t += batch_tiles
        self.f0, self.f1, self.f2 = 64, 128, 64


# ---------------------------------------------------------------------------
# Host-side preprocessing (graph partitioning / normalization structure)
# ---------------------------------------------------------------------------


def prepare(x, edge_index, W1, b1, W2, b2, n_cores=8, half=32768, batch_tiles=2):
    x = np.asarray(x, dtype=np.float32)
    edge_index = np.asarray(edge_index)
    W1 = np.asarray(W1, dtype=np.float32)
    b1 = np.asarray(b1, dtype=np.float32)
    W2 = np.asarray(W2, dtype=np.float32)
    b2 = np.asarray(b2, dtype=np.float32)

    n = x.shape[0]
    src = edge_index[0].astype(np.int64)
    dst = edge_index[1].astype(np.int64)

    deg = 1.0 + np.bincount(dst, minlength=n).astype(np.float64)
    dis = (1.0 / np.sqrt(deg)).astype(np.float32)

    # self loops as ordinary edges
    loops = np.arange(n, dtype=np.int64)
    src_all = np.concatenate([src, loops])
    dst_all = np.concatenate([dst, loops])
    norm_all = dis[src_all] * dis[dst_all]

    npc = n // n_cores
    core_of = dst_all // npc

    # pass 1: per-(core, tile, half) counts -> global uniform chunk counts
    per_core = []
    c0_max = 1
    c1_max = 1
    nt = (npc + 127) // 128
    for c in range(n_cores):
        m = core_of == c
        s = src_all[m]
        d = dst_all[m] - c * npc
        nr = norm_all[m]
        tile_id = d >> 7
        h = (s >= half).astype(np.int64)
        key = tile_id * 2 + h
        order = np.argsort(key, kind="stable")
        sk = key[order]
        bounds = np.searchsorted(sk, np.arange(nt * 2 + 1))
        cnt = np.diff(bounds)
        c0_max = max(c0_max, int(math.ceil(cnt[0::2].max() / 128.0)))
        c1_max = max(c1_max, int(math.ceil(cnt[1::2].max() / 128.0)))
        per_core.append((s, d, nr, order, bounds))

    cfg = Cfg(n, n_cores, half, batch_tiles, c0_max, c1_max)

    # pass 2: build padded streams in batch order
    maps = []
    for c in range(n_cores):
        s, d, nr, order, bounds = per_core[c]
        idx_stream = np.zeros(cfg.nchunk * 128, np.int16)
        dstl_stream = np.full(cfg.nchunk * 128, 500.0, np.float32)
        norm_stream = np.zeros(cfg.nchunk * 128, np.float32)
        pos = 0  # in chunks
        for batch in cfg.batches:
            for hh, cmax in ((0, cfg.c0), (1, cfg.c1)):
                for t in batch:
                    seg = order[bounds[2 * t + hh]:bounds[2 * t + hh + 1]]
                    L = len(seg)
                    base = pos * 128
                    sv = s[seg]
                    if hh:
                        sv = sv - half
                    idx_stream[base:base + L] = sv.astype(np.int16)
                    dstl_stream[base:base + L] = (d[seg] & 127).astype(np.float32)
                    norm_stream[base:base + L] = nr[seg]
                    pos += cmax
        assert pos == cfg.nchunk
        eidx = np.tile(idx_stream.reshape(cfg.nchunk * 8, 16).T, (8, 1))
        edstl = np.ascontiguousarray(dstl_stream.reshape(cfg.nchunk, 128).T)
        enorm = np.ascontiguousarray(norm_stream.reshape(cfg.nchunk, 128).T)
        maps.append({"eidx": eidx, "edstl": edstl, "enorm": enorm})

    xt = np.zeros((n, 128), np.float16)
    xt[:, :64] = x
    shared = {
        "xt": xt,
        "w1": W1.astype(np.float16),
        "b1": np.ascontiguousarray(b1.reshape(cfg.f1, 1)),
        "w2": W2.astype(np.float16),
        "b2": np.ascontiguousarray(b2.reshape(cfg.f2, 1)),
        "iota": np.tile(np.arange(128, dtype=np.float16), (128, 1)),
        "idA": np.eye(128, dtype=np.float16),
        "idB": np.eye(cfg.f2, dtype=np.float32),
    }
    in_maps = [{**shared, **m} for m in maps]
    return cfg, in_maps


# ---------------------------------------------------------------------------
# Device program
# ---------------------------------------------------------------------------


def build_program(cfg):
    import concourse.bacc as bacc
    import concourse.mybir as mybir
    import concourse.tile as tile

    dt = mybir.dt
    AF = mybir.ActivationFunctionType
    ALU = mybir.AluOpType

    n, npc, nt, half = cfg.n, cfg.npc, cfg.nt, cfg.half
    c0, c1 = cfg.c0, cfg.c1
    nchunk = cfg.nchunk
    F0, F1, F2 = cfg.f0, cfg.f1, cfg.f2

    nc = bacc.Bacc(
        "TRN2",
        target_bir_lowering=False,
        debug=False,
        enable_asserts=False,
        num_devices=cfg.n_cores,
        num_swdge_queues=4,
    )

    xt = nc.dram_tensor("xt", [n, 128], dt.float16, kind="ExternalInput")
    eidx = nc.dram_tensor("eidx", [128, nchunk * 8], dt.int16, kind="ExternalInput")
    edstl = nc.dram_tensor("edstl", [128, nchunk], dt.float32, kind="ExternalInput")
    enorm = nc.dram_tensor("enorm", [128, nchunk], dt.float32, kind="ExternalInput")
    w1t = nc.dram_tensor("w1", [F0, F1], dt.float16, kind="ExternalInput")
    b1t = nc.dram_tensor("b1", [F1, 1], dt.float32, kind="ExternalInput")
    w2t = nc.dram_tensor("w2", [F1, F2], dt.float16, kind="ExternalInput")
    b2t = nc.dram_tensor("b2", [F2, 1], dt.float32, kind="ExternalInput")
    iotat = nc.dram_tensor("iota", [128, 128], dt.float16, kind="ExternalInput")
    idAt = nc.dram_tensor("idA", [128, 128], dt.float16, kind="ExternalInput")
    idBt = nc.dram_tensor("idB", [F2, F2], dt.float32, kind="ExternalInput")
    outt = nc.dram_tensor("out", [npc, F2], dt.float32, kind="ExternalOutput")

    with tile.TileContext(nc) as tc:
        with (
            tc.tile_pool(name="const", bufs=1) as cp,
            tc.tile_pool(name="edges", bufs=1) as ep,
            tc.tile_pool(name="gb", bufs=3) as gp,
            tc.tile_pool(name="v", bufs=8) as vp,
            tc.tile_pool(name="sb", bufs=3) as sp,
            tc.tile_pool(name="psA", bufs=2, space="PSUM") as psA,
            tc.tile_pool(name="psZ", bufs=2, space="PSUM") as psZ,
            tc.tile_pool(name="psT", bufs=2, space="PSUM") as psT,
            tc.tile_pool(name="dram", bufs=1, space="DRAM") as dp,
        ):
            def load_const(pool, t, dtype):
                sb = pool.tile(list(t.shape), dtype, tag=t.name)
                nc.sync.dma_start(sb[:], t.ap()[:])
                return sb

            iota_sb = load_const(cp, iotat, dt.float16)
            idA_sb = load_const(cp, idAt, dt.float16)
            idB_sb = load_const(cp, idBt, dt.float32)
            w1_sb = load_const(cp, w1t, dt.float16)
            b1_sb = load_const(cp, b1t, dt.float32)
            w2_sb = load_const(cp, w2t, dt.float16)
            b2_sb = load_const(cp, b2t, dt.float32)
            eidx_sb = load_const(ep, eidx, dt.int16)
            edstl_sb = load_const(ep, edstl, dt.float32)
            enorm_sb = load_const(ep, enorm, dt.float32)

            cc_in = dp.tile([npc, F1], dt.float16)
            cc_out = dp.tile([n, F1], dt.float16)

            n_regs = {}
            for bt in {len(b) for b in cfg.batches}:
                n_regs[bt * c0 * 128] = nc.gpsimd.to_reg(bt * c0 * 128)
                n_regs[bt * c1 * 128] = nc.gpsimd.to_reg(bt * c1 * 128)

            def do_layer(table, f_in, w_sb, b_sb, out_dt, id_sb, f_out, write_out, qoff):
                g_base = 0
                for bi, batch in enumerate(cfg.batches):
                    bt = len(batch)
                    nch = bt * (c0 + c1)
                    gb = gp.tile([128, nch, 128], dt.float16, tag="gb")
                    n0 = bt * c0 * 128
                    n1 = bt * c1 * 128
                    nc.gpsimd.dma_gather(
                        gb[:, 0:bt * c0, :],
                        table[0:half, :],
                        eidx_sb[:, g_base * 8:(g_base + bt * c0) * 8],
                        num_idxs=n0,
                        num_idxs_reg=n_regs[n0],
                        elem_size=128,
                        single_packet=False,
                        queue_num=(qoff + 2 * bi) % 4,
                    )
                    nc.gpsimd.dma_gather(
                        gb[:, bt * c0:nch, :],
                        table[half:n, :],
                        eidx_sb[:, (g_base + bt * c0) * 8:(g_base + nch) * 8],
                        num_idxs=n1,
                        num_idxs_reg=n_regs[n1],
                        elem_size=128,
                        single_packet=False,
                        queue_num=(qoff + 2 * bi + 1) % 4,
                    )
                    for i, t in enumerate(batch):
                        agg_ps = psA.tile([f_in, 128], dt.float32, tag="psA")
                        slots = list(range(i * c0, (i + 1) * c0)) + list(
                            range(bt * c0 + i * c1, bt * c0 + (i + 1) * c1)
                        )
                        for j, s in enumerate(slots):
                            g = g_base + s
                            V = vp.tile([128, 128], dt.float16, tag="v")
                            nc.vector.tensor_scalar(
                                V[:],
                                iota_sb[:],
                                edstl_sb[:, g:g + 1],
                                enorm_sb[:, g:g + 1],
                                ALU.is_equal,
                                ALU.mult,
                            )
                            nc.tensor.matmul(
                                agg_ps[:],
                                gb[:, s, 0:f_in],
                                V[:],
                                start=(j == 0),
                                stop=(j == len(slots) - 1),
                            )
                        agg_sb = sp.tile([f_in, 128], dt.float16, tag="agg")
                        nc.vector.tensor_copy(agg_sb[:], agg_ps[:])
                        z_ps = psZ.tile([f_out, 128], dt.float32, tag="psZ")
                        nc.tensor.matmul(z_ps[:], w_sb[:], agg_sb[:], start=True, stop=True)
                        zr_sb = sp.tile([f_out, 128], out_dt, tag="zr")
                        nc.scalar.activation(zr_sb[:], z_ps[:], AF.Relu, bias=b_sb[:], scale=1.0)
                        tr_ps = psT.tile([128, f_out], out_dt, tag="psT")
                        nc.tensor.transpose(tr_ps[:], zr_sb[:], id_sb[:])
                        h_sb = sp.tile([128, f_out], out_dt, tag="h")
                        nc.vector.tensor_copy(h_sb[:], tr_ps[:])
                        rows = cfg.last_rows if t == nt - 1 else 128
                        write_out(t, h_sb, rows)
                    g_base += nch

            def w1_out(t, h_sb, rows):
                nc.sync.dma_start(cc_in[t * 128:t * 128 + rows, :], h_sb[0:rows, :])

            do_layer(xt.ap(), F0, w1_sb, b1_sb, dt.float16, idA_sb, F1, w1_out, 0)

            nc.gpsimd.collective_compute(
                "AllGather",
                ALU.bypass,
                replica_groups=[list(range(cfg.n_cores))],
                ins=[cc_in.opt()],
                outs=[cc_out.opt()],
            )

            def w2_out(t, h_sb, rows):
                nc.sync.dma_start(outt.ap()[t * 128:t * 128 + rows, :], h_sb[0:rows, :])

            do_layer(cc_out[:, :], F1, w2_sb, b2_sb, dt.float32, idB_sb, F2, w2_out, 2)

    nc.compile()
    return nc


# ---------------------------------------------------------------------------
# Entry point
# ---------------------------------------------------------------------------

_CACHE = {}


def kernel(x, edge_index, W1, b1, W2, b2):
    x = np.asarray(x)
    cfg, in_maps = prepare(x, edge_index, W1, b1, W2, b2)

    key = (cfg.n, cfg.n_cores, cfg.c0, cfg.c1)
    nc = _CACHE.get(key)
    if nc is None:
        nc = build_program(cfg)
        _CACHE[key] = nc

    from concourse.bass_utils import run_bass_kernel_spmd

    res = run_bass_kernel_spmd(nc, in_maps, core_ids=list(range(cfg.n_cores)))
    out = np.concatenate([r["out"] for r in res.results], axis=0)
    return np.ascontiguousarray(out.astype(np.float32))


# revision 5
# speedup vs baseline: 1.4383x; 1.0344x over previous
"""Two-layer GCN (PyG GCNConv x2 + ReLU) on 8 Trainium2 NeuronCores.

Math: out = relu(S @ (relu(S @ (x W1) + b1) W2) + b2), with
S = D^-1/2 (A + I) D^-1/2 the symmetric-normalized adjacency (1.6M random
edges + self loops over 50000 nodes).

Key reformulation: aggregation is linear, so S (x W) == (S x) W.  Both
layers aggregate in the *small* (64-wide) feature space:
  layer1: agg = S x            (64 wide), h1 = relu(agg @ W1 + b1)
  layer2: agg = S (h1 W2)?  -> actually h1 is 128 wide; we aggregate h1
          directly (128 wide) and transform after: relu((S h1) @ W2 + b2).

Sharding: destination nodes are range-partitioned across the 8 cores
(6250 per core).  Each core owns the edges whose dst falls in its range
(plus its self-loops), pre-sorted by dst tile on the host.  Layer-1's
gather table is the (replicated) input x itself; layer-2's table is the
AllGather of the per-core h1 shards.  Weights are replicated.

Per 128-edge chunk the device does:
  - dma_gather: 128 rows of the node table -> SBUF [128 edges, 128 feats]
  - DVE tensor_scalar builds V[e,d] = (iota[d]==dstl[e]) * norm[e]
    (norm = dis[src]*dis[dst] precomputed per edge on the host; self
    loops are ordinary edges with norm = dis^2)
  - PE matmul accumulates agg.T[f,d] += gathered.T @ V in PSUM
Per 128-dst tile: transform with W (PE), bias+relu (ACT, bias is
per-partition in the transposed layout), transpose back to node-major
(PE), and DMA out.
"""

import math

import numpy as np

# ---------------------------------------------------------------------------
# Configuration
# ---------------------------------------------------------------------------


class Cfg:
    def __init__(self, n, n_cores, half, batch_tiles, c0, c1):
        self.n = n                       # total nodes
        self.n_cores = n_cores
        self.npc = n // n_cores          # nodes (dsts) per core
        self.nt = (self.npc + 127) // 128  # dst tiles per core
        self.last_rows = self.npc - (self.nt - 1) * 128
        self.half = half                 # table split point (int16 idx range)
        self.c0 = c0                     # chunks per tile, src < half
        self.c1 = c1                     # chunks per tile, src >= half
        self.nchunk = self.nt * (c0 + c1)
        self.batches = []
        t = 0
        while t < self.nt:
            self.batches.append(list(range(t, min(t + batch_tiles, self.nt))))
            t += batch_tiles
        self.f0, self.f1, self.f2 = 64, 128, 64


# ---------------------------------------------------------------------------
# Host-side preprocessing (graph partitioning / normalization structure)
# ---------------------------------------------------------------------------


def prepare(x, edge_index, W1, b1, W2, b2, n_cores=8, half=32768, batch_tiles=1):
    x = np.asarray(x, dtype=np.float32)
    edge_index = np.asarray(edge_index)
    W1 = np.asarray(W1, dtype=np.float32)
    b1 = np.asarray(b1, dtype=np.float32)
    W2 = np.asarray(W2, dtype=np.float32)
    b2 = np.asarray(b2, dtype=np.float32)

    n = x.shape[0]
    src = edge_index[0].astype(np.int64)
    dst = edge_index[1].astype(np.int64)

    deg = 1.0 + np.bincount(dst, minlength=n).astype(np.float64)
    dis = (1.0 / np.sqrt(deg)).astype(np.float32)

    # self loops as ordinary edges
    loops = np.arange(n, dtype=np.int64)
    src_all = np.concatenate([src, loops])
    dst_all = np.concatenate([dst, loops])
    norm_all = dis[src_all] * dis[dst_all]

    npc = n // n_cores
    core_of = dst_all // npc

    # pass 1: per-(core, tile, half) counts -> global uniform chunk counts
    per_core = []
    c0_max = 1
    c1_max = 1
    nt = (npc + 127) // 128
    for c in range(n_cores):
        m = core_of == c
        s = src_all[m]
        d = dst_all[m] - c * npc
        nr = norm_all[m]
        tile_id = d >> 7
        h = (s >= half).astype(np.int64)
        key = tile_id * 2 + h
        order = np.argsort(key, kind="stable")
        sk = key[order]
        bounds = np.searchsorted(sk, np.arange(nt * 2 + 1))
        cnt = np.diff(bounds)
        c0_max = max(c0_max, int(math.ceil(cnt[0::2].max() / 128.0)))
        c1_max = max(c1_max, int(math.ceil(cnt[1::2].max() / 128.0)))
        per_core.append((s, d, nr, order, bounds))

    cfg = Cfg(n, n_cores, half, batch_tiles, c0_max, c1_max)

    # pass 2: build padded streams in batch order
    maps = []
    for c in range(n_cores):
        s, d, nr, order, bounds = per_core[c]
        idx_stream = np.zeros(cfg.nchunk * 128, np.int16)
        dstl_stream = np.full(cfg.nchunk * 128, 500.0, np.float32)
        norm_stream = np.zeros(cfg.nchunk * 128, np.float32)
        pos = 0  # in chunks
        for batch in cfg.batches:
            for hh, cmax in ((0, cfg.c0), (1, cfg.c1)):
                for t in batch:
                    seg = order[bounds[2 * t + hh]:bounds[2 * t + hh + 1]]
                    L = len(seg)
                    base = pos * 128
                    sv = s[seg]
                    if hh:
                        sv = sv - half
                    idx_stream[base:base + L] = sv.astype(np.int16)
                    dstl_stream[base:base + L] = (d[seg] & 127).astype(np.float32)
                    norm_stream[base:base + L] = nr[seg]
                    pos += cmax
        assert pos == cfg.nchunk
        eidx = np.tile(idx_stream.reshape(cfg.nchunk * 8, 16).T, (8, 1))
        edstl = np.ascontiguousarray(dstl_stream.reshape(cfg.nchunk, 128).T)
        enorm = np.ascontiguousarray(norm_stream.reshape(cfg.nchunk, 128).T)
        maps.append({"eidx": eidx, "edstl": edstl, "enorm": enorm})

    xt = np.zeros((n, 128), np.float16)
    xt[:, :64] = x
    shared = {
        "xt": xt,
        "w1": W1.astype(np.float16),
        "b1": np.ascontiguousarray(b1.reshape(cfg.f1, 1)),
        "w2": W2.astype(np.float16),
        "b2": np.ascontiguousarray(b2.reshape(cfg.f2, 1)),
        "iota": np.tile(np.arange(128, dtype=np.float16), (128, 1)),
        "idA": np.eye(128, dtype=np.float16),
        "idB": np.eye(cfg.f2, dtype=np.float32),
    }
    in_maps = [{**shared, **m} for m in maps]
    return cfg, in_maps


# ---------------------------------------------------------------------------
# Device program
# ---------------------------------------------------------------------------


def build_program(cfg):
    import concourse.bacc as bacc
    import concourse.mybir as mybir
    import concourse.tile as tile

    dt = mybir.dt
    AF = mybir.ActivationFunctionType
    ALU = mybir.AluOpType

    n, npc, nt, half = cfg.n, cfg.npc, cfg.nt, cfg.half
    c0, c1 = cfg.c0, cfg.c1
    nchunk = cfg.nchunk
    F0, F1, F2 = cfg.f0, cfg.f1, cfg.f2

    nc = bacc.Bacc(
        "TRN2",
        target_bir_lowering=False,
        debug=False,
        enable_asserts=False,
        num_devices=cfg.n_cores,
        num_swdge_queues=4,
    )

    xt = nc.dram_tensor("xt", [n, 128], dt.float16, kind="ExternalInput")
    eidx = nc.dram_tensor("eidx", [128, nchunk * 8], dt.int16, kind="ExternalInput")
    edstl = nc.dram_tensor("edstl", [128, nchunk], dt.float32, kind="ExternalInput")
    enorm = nc.dram_tensor("enorm", [128, nchunk], dt.float32, kind="ExternalInput")
    w1t = nc.dram_tensor("w1", [F0, F1], dt.float16, kind="ExternalInput")
    b1t = nc.dram_tensor("b1", [F1, 1], dt.float32, kind="ExternalInput")
    w2t = nc.dram_tensor("w2", [F1, F2], dt.float16, kind="ExternalInput")
    b2t = nc.dram_tensor("b2", [F2, 1], dt.float32, kind="ExternalInput")
    iotat = nc.dram_tensor("iota", [128, 128], dt.float16, kind="ExternalInput")
    idAt = nc.dram_tensor("idA", [128, 128], dt.float16, kind="ExternalInput")
    idBt = nc.dram_tensor("idB", [F2, F2], dt.float32, kind="ExternalInput")
    outt = nc.dram_tensor("out", [npc, F2], dt.float32, kind="ExternalOutput")

    with tile.TileContext(nc) as tc:
        with (
            tc.tile_pool(name="const", bufs=1) as cp,
            tc.tile_pool(name="edges", bufs=1) as ep,
            tc.tile_pool(name="gb", bufs=6) as gp,
            tc.tile_pool(name="v", bufs=12) as vp,
            tc.tile_pool(name="sb", bufs=3) as sp,
            tc.tile_pool(name="psA", bufs=2, space="PSUM") as psA,
            tc.tile_pool(name="psZ", bufs=2, space="PSUM") as psZ,
            tc.tile_pool(name="psT", bufs=2, space="PSUM") as psT,
            tc.tile_pool(name="dram", bufs=1, space="DRAM") as dp,
        ):
            def load_const(pool, t, dtype):
                sb = pool.tile(list(t.shape), dtype, tag=t.name)
                nc.sync.dma_start(sb[:], t.ap()[:])
                return sb

            iota_sb = load_const(cp, iotat, dt.float16)
            idA_sb = load_const(cp, idAt, dt.float16)
            idB_sb = load_const(cp, idBt, dt.float32)
            w1_sb = load_const(cp, w1t, dt.float16)
            b1_sb = load_const(cp, b1t, dt.float32)
            w2_sb = load_const(cp, w2t, dt.float16)
            b2_sb = load_const(cp, b2t, dt.float32)
            eidx_sb = load_const(ep, eidx, dt.int16)
            edstl_sb = load_const(ep, edstl, dt.float32)
            enorm_sb = load_const(ep, enorm, dt.float32)

            cc_in = dp.tile([npc, F1], dt.float16)
            cc_out = dp.tile([n, F1], dt.float16)

            n_regs = {}
            for bt in {len(b) for b in cfg.batches}:
                n_regs[bt * c0 * 128] = nc.gpsimd.to_reg(bt * c0 * 128)
                n_regs[bt * c1 * 128] = nc.gpsimd.to_reg(bt * c1 * 128)

            def do_layer(table, f_in, w_sb, b_sb, out_dt, id_sb, f_out, write_out, qoff):
                g_base = 0
                for bi, batch in enumerate(cfg.batches):
                    bt = len(batch)
                    nch = bt * (c0 + c1)
                    gb = gp.tile([128, nch, 128], dt.float16, tag="gb")
                    n0 = bt * c0 * 128
                    n1 = bt * c1 * 128
                    nc.gpsimd.dma_gather(
                        gb[:, 0:bt * c0, :],
                        table[0:half, :],
                        eidx_sb[:, g_base * 8:(g_base + bt * c0) * 8],
                        num_idxs=n0,
                        num_idxs_reg=n_regs[n0],
                        elem_size=128,
                        single_packet=False,
                        queue_num=(qoff + 2 * bi) % 4,
                    )
                    nc.gpsimd.dma_gather(
                        gb[:, bt * c0:nch, :],
                        table[half:n, :],
                        eidx_sb[:, (g_base + bt * c0) * 8:(g_base + nch) * 8],
                        num_idxs=n1,
                        num_idxs_reg=n_regs[n1],
                        elem_size=128,
                        single_packet=False,
                        queue_num=(qoff + 2 * bi + 1) % 4,
                    )
                    for i, t in enumerate(batch):
                        agg_ps = psA.tile([f_in, 128], dt.float32, tag="psA")
                        slots = list(range(i * c0, (i + 1) * c0)) + list(
                            range(bt * c0 + i * c1, bt * c0 + (i + 1) * c1)
                        )
                        for j, s in enumerate(slots):
                            g = g_base + s
                            V = vp.tile([128, 128], dt.float16, tag="v")
                            nc.vector.tensor_scalar(
                                V[:],
                                iota_sb[:],
                                edstl_sb[:, g:g + 1],
                                enorm_sb[:, g:g + 1],
                                ALU.is_equal,
                                ALU.mult,
                            )
                            nc.tensor.matmul(
                                agg_ps[:],
                                gb[:, s, 0:f_in],
                                V[:],
                                start=(j == 0),
                                stop=(j == len(slots) - 1),
                            )
                        agg_sb = sp.tile([f_in, 128], dt.float16, tag="agg")
                        nc.scalar.copy(agg_sb[:], agg_ps[:])
                        z_ps = psZ.tile([f_out, 128], dt.float32, tag="psZ")
                        nc.tensor.matmul(z_ps[:], w_sb[:], agg_sb[:], start=True, stop=True)
                        zr_sb = sp.tile([f_out, 128], out_dt, tag="zr")
                        nc.scalar.activation(zr_sb[:], z_ps[:], AF.Relu, bias=b_sb[:], scale=1.0)
                        tr_ps = psT.tile([128, f_out], out_dt, tag="psT")
                        nc.tensor.transpose(tr_ps[:], zr_sb[:], id_sb[:])
                        h_sb = sp.tile([128, f_out], out_dt, tag="h")
                        nc.scalar.copy(h_sb[:], tr_ps[:])
                        rows = cfg.last_rows if t == nt - 1 else 128
                        write_out(t, h_sb, rows)
                    g_base += nch

            def w1_out(t, h_sb, rows):
                nc.sync.dma_start(cc_in[t * 128:t * 128 + rows, :], h_sb[0:rows, :])

            do_layer(xt.ap(), F0, w1_sb, b1_sb, dt.float16, idA_sb, F1, w1_out, 0)

            nc.gpsimd.collective_compute(
                "AllGather",
                ALU.bypass,
                replica_groups=[list(range(cfg.n_cores))],
                ins=[cc_in.opt()],
                outs=[cc_out.opt()],
            )

            def w2_out(t, h_sb, rows):
                nc.sync.dma_start(outt.ap()[t * 128:t * 128 + rows, :], h_sb[0:rows, :])

            do_layer(cc_out[:, :], F1, w2_sb, b2_sb, dt.float32, idB_sb, F2, w2_out, 2)

    nc.compile()
    return nc


# ---------------------------------------------------------------------------
# Entry point
# ---------------------------------------------------------------------------

_CACHE = {}


def kernel(x, edge_index, W1, b1, W2, b2):
    x = np.asarray(x)
    cfg, in_maps = prepare(x, edge_index, W1, b1, W2, b2)

    key = (cfg.n, cfg.n_cores, cfg.c0, cfg.c1)
    nc = _CACHE.get(key)
    if nc is None:
        nc = build_program(cfg)
        _CACHE[key] = nc

    from concourse.bass_utils import run_bass_kernel_spmd

    res = run_bass_kernel_spmd(nc, in_maps, core_ids=list(range(cfg.n_cores)))
    out = np.concatenate([r["out"] for r in res.results], axis=0)
    return np.ascontiguousarray(out.astype(np.float32))


# revision 6
# speedup vs baseline: 1.5635x; 1.0870x over previous
"""Two-layer GCN (PyG GCNConv x2 + ReLU) on 8 Trainium2 NeuronCores.

Math: out = relu(S @ (relu(S @ (x W1) + b1) W2) + b2), with
S = D^-1/2 (A + I) D^-1/2 the symmetric-normalized adjacency (1.6M random
edges + self loops over 50000 nodes).

Key reformulation: aggregation is linear, so S (x W) == (S x) W.  Both
layers aggregate in the *small* (64-wide) feature space:
  layer1: agg = S x            (64 wide), h1 = relu(agg @ W1 + b1)
  layer2: agg = S (h1 W2)?  -> actually h1 is 128 wide; we aggregate h1
          directly (128 wide) and transform after: relu((S h1) @ W2 + b2).

Sharding: destination nodes are range-partitioned across the 8 cores
(6250 per core).  Each core owns the edges whose dst falls in its range
(plus its self-loops), pre-sorted by dst tile on the host.  Layer-1's
gather table is the (replicated) input x itself; layer-2's table is the
AllGather of the per-core h1 shards.  Weights are replicated.

Per 128-edge chunk the device does:
  - dma_gather: 128 rows of the node table -> SBUF [128 edges, 128 feats]
  - DVE tensor_scalar builds V[e,d] = (iota[d]==dstl[e]) * norm[e]
    (norm = dis[src]*dis[dst] precomputed per edge on the host; self
    loops are ordinary edges with norm = dis^2)
  - PE matmul accumulates agg.T[f,d] += gathered.T @ V in PSUM
Per 128-dst tile: transform with W (PE), bias+relu (ACT, bias is
per-partition in the transposed layout), transpose back to node-major
(PE), and DMA out.
"""

import math

import numpy as np

# ---------------------------------------------------------------------------
# Configuration
# ---------------------------------------------------------------------------


class Cfg:
    def __init__(self, n, n_cores, half, batch_tiles, c0, c1):
        self.n = n                       # total nodes
        self.n_cores = n_cores
        self.npc = n // n_cores          # nodes (dsts) per core
        self.nt = (self.npc + 127) // 128  # dst tiles per core
        self.last_rows = self.npc - (self.nt - 1) * 128
        self.half = half                 # table split point (int16 idx range)
        self.c0 = c0                     # chunks per tile, src < half
        self.c1 = c1                     # chunks per tile, src >= half
        self.nchunk = self.nt * (c0 + c1)
        self.batches = []
        t = 0
        while t < self.nt:
            self.batches.append(list(range(t, min(t + batch_tiles, self.nt))))
            t += batch_tiles
        self.f0, self.f1, self.f2 = 64, 128, 64


# ---------------------------------------------------------------------------
# Host-side preprocessing (graph partitioning / normalization structure)
# ---------------------------------------------------------------------------


def prepare(x, edge_index, W1, b1, W2, b2, n_cores=8, half=32768, batch_tiles=1):
    x = np.asarray(x, dtype=np.float32)
    edge_index = np.asarray(edge_index)
    W1 = np.asarray(W1, dtype=np.float32)
    b1 = np.asarray(b1, dtype=np.float32)
    W2 = np.asarray(W2, dtype=np.float32)
    b2 = np.asarray(b2, dtype=np.float32)

    n = x.shape[0]
    src = edge_index[0].astype(np.int64)
    dst = edge_index[1].astype(np.int64)

    deg = 1.0 + np.bincount(dst, minlength=n).astype(np.float64)
    dis = (1.0 / np.sqrt(deg)).astype(np.float32)

    # self loops as ordinary edges
    loops = np.arange(n, dtype=np.int64)
    src_all = np.concatenate([src, loops])
    dst_all = np.concatenate([dst, loops])
    norm_all = dis[src_all] * dis[dst_all]

    npc = n // n_cores
    core_of = dst_all // npc

    # pass 1: per-(core, tile, half) counts -> global uniform chunk counts
    per_core = []
    c0_max = 1
    c1_max = 1
    nt = (npc + 127) // 128
    for c in range(n_cores):
        m = core_of == c
        s = src_all[m]
        d = dst_all[m] - c * npc
        nr = norm_all[m]
        tile_id = d >> 7
        h = (s >= half).astype(np.int64)
        key = tile_id * 2 + h
        order = np.argsort(key, kind="stable")
        sk = key[order]
        bounds = np.searchsorted(sk, np.arange(nt * 2 + 1))
        cnt = np.diff(bounds)
        c0_max = max(c0_max, int(math.ceil(cnt[0::2].max() / 128.0)))
        c1_max = max(c1_max, int(math.ceil(cnt[1::2].max() / 128.0)))
        per_core.append((s, d, nr, order, bounds))

    cfg = Cfg(n, n_cores, half, batch_tiles, c0_max, c1_max)

    # pass 2: build padded streams in batch order
    maps = []
    for c in range(n_cores):
        s, d, nr, order, bounds = per_core[c]
        idx_stream = np.zeros(cfg.nchunk * 128, np.int16)
        dstl_stream = np.full(cfg.nchunk * 128, 500.0, np.float32)
        norm_stream = np.zeros(cfg.nchunk * 128, np.float32)
        pos = 0  # in chunks
        for batch in cfg.batches:
            for hh, cmax in ((0, cfg.c0), (1, cfg.c1)):
                for t in batch:
                    seg = order[bounds[2 * t + hh]:bounds[2 * t + hh + 1]]
                    L = len(seg)
                    base = pos * 128
                    sv = s[seg]
                    if hh:
                        sv = sv - half
                    idx_stream[base:base + L] = sv.astype(np.int16)
                    dstl_stream[base:base + L] = (d[seg] & 127).astype(np.float32)
                    norm_stream[base:base + L] = nr[seg]
                    pos += cmax
        assert pos == cfg.nchunk
        eidx = np.tile(idx_stream.reshape(cfg.nchunk * 8, 16).T, (8, 1))
        edstl = np.ascontiguousarray(dstl_stream.reshape(cfg.nchunk, 128).T)
        enorm = np.ascontiguousarray(norm_stream.reshape(cfg.nchunk, 128).T)
        maps.append({"eidx": eidx, "edstl": edstl, "enorm": enorm})

    xt = np.zeros((n, 128), np.float16)
    xt[:, :64] = x
    shared = {
        "xt": xt,
        "w1": W1.astype(np.float16),
        "b1": np.ascontiguousarray(b1.reshape(cfg.f1, 1)),
        "w2": W2.astype(np.float16),
        "b2": np.ascontiguousarray(b2.reshape(cfg.f2, 1)),
        "iota": np.tile(np.arange(128, dtype=np.float16), (128, 1)),
        "idA": np.eye(128, dtype=np.float16),
        "idB": np.eye(cfg.f2, dtype=np.float32),
    }
    in_maps = [{**shared, **m} for m in maps]
    return cfg, in_maps


# ---------------------------------------------------------------------------
# Device program
# ---------------------------------------------------------------------------


def build_program(cfg):
    import concourse.bacc as bacc
    import concourse.mybir as mybir
    import concourse.tile as tile

    dt = mybir.dt
    AF = mybir.ActivationFunctionType
    ALU = mybir.AluOpType

    n, npc, nt, half = cfg.n, cfg.npc, cfg.nt, cfg.half
    c0, c1 = cfg.c0, cfg.c1
    nchunk = cfg.nchunk
    F0, F1, F2 = cfg.f0, cfg.f1, cfg.f2

    nc = bacc.Bacc(
        "TRN2",
        target_bir_lowering=False,
        debug=False,
        enable_asserts=False,
        num_devices=cfg.n_cores,
        num_swdge_queues=4,
    )

    xt = nc.dram_tensor("xt", [n, 128], dt.float16, kind="ExternalInput")
    eidx = nc.dram_tensor("eidx", [128, nchunk * 8], dt.int16, kind="ExternalInput")
    edstl = nc.dram_tensor("edstl", [128, nchunk], dt.float32, kind="ExternalInput")
    enorm = nc.dram_tensor("enorm", [128, nchunk], dt.float32, kind="ExternalInput")
    w1t = nc.dram_tensor("w1", [F0, F1], dt.float16, kind="ExternalInput")
    b1t = nc.dram_tensor("b1", [F1, 1], dt.float32, kind="ExternalInput")
    w2t = nc.dram_tensor("w2", [F1, F2], dt.float16, kind="ExternalInput")
    b2t = nc.dram_tensor("b2", [F2, 1], dt.float32, kind="ExternalInput")
    iotat = nc.dram_tensor("iota", [128, 128], dt.float16, kind="ExternalInput")
    idAt = nc.dram_tensor("idA", [128, 128], dt.float16, kind="ExternalInput")
    idBt = nc.dram_tensor("idB", [F2, F2], dt.float32, kind="ExternalInput")
    outt = nc.dram_tensor("out", [npc, F2], dt.float32, kind="ExternalOutput")

    with tile.TileContext(nc) as tc:
        with (
            tc.tile_pool(name="const", bufs=1) as cp,
            tc.tile_pool(name="edges", bufs=1) as ep,
            tc.tile_pool(name="gb", bufs=4) as gp,
            tc.tile_pool(name="lb", bufs=3) as lp,
            tc.tile_pool(name="v", bufs=12) as vp,
            tc.tile_pool(name="sb", bufs=3) as sp,
            tc.tile_pool(name="psA", bufs=2, space="PSUM") as psA,
            tc.tile_pool(name="psZ", bufs=2, space="PSUM") as psZ,
            tc.tile_pool(name="psT", bufs=2, space="PSUM") as psT,
            tc.tile_pool(name="dram", bufs=1, space="DRAM") as dp,
        ):
            def load_const(pool, t, dtype):
                sb = pool.tile(list(t.shape), dtype, tag=t.name)
                nc.sync.dma_start(sb[:], t.ap()[:])
                return sb

            iota_sb = load_const(cp, iotat, dt.float16)
            idA_sb = load_const(cp, idAt, dt.float16)
            idB_sb = load_const(cp, idBt, dt.float32)
            w1_sb = load_const(cp, w1t, dt.float16)
            b1_sb = load_const(cp, b1t, dt.float32)
            w2_sb = load_const(cp, w2t, dt.float16)
            b2_sb = load_const(cp, b2t, dt.float32)
            eidx_sb = load_const(ep, eidx, dt.int16)
            edstl_sb = load_const(ep, edstl, dt.float32)
            enorm_sb = load_const(ep, enorm, dt.float32)

            cc_in = dp.tile([npc, F1], dt.float16)
            cc_out = dp.tile([n, F1], dt.float16)
            stage1 = dp.tile([128, nchunk, 128], dt.float16)
            stage2 = dp.tile([128, nchunk, 128], dt.float16)

            n_regs = {}
            for bt in {len(b) for b in cfg.batches}:
                n_regs[bt * c0 * 128] = nc.gpsimd.to_reg(bt * c0 * 128)
                n_regs[bt * c1 * 128] = nc.gpsimd.to_reg(bt * c1 * 128)

            def do_layer(table, f_in, w_sb, b_sb, out_dt, id_sb, f_out, write_out,
                         qoff, stage):
                # phase A: gathers stream into SBUF and are immediately staged
                # out to DRAM; slots recycle fast so desc-gen pipelines across
                # the 4 SWDGE queue core-pairs without waiting on compute.
                g_base = 0
                for bi, batch in enumerate(cfg.batches):
                    bt = len(batch)
                    nch = bt * (c0 + c1)
                    gb = gp.tile([128, nch, 128], dt.float16, tag="gb")
                    n0 = bt * c0 * 128
                    n1 = bt * c1 * 128
                    nc.gpsimd.dma_gather(
                        gb[:, 0:bt * c0, :],
                        table[0:half, :],
                        eidx_sb[:, g_base * 8:(g_base + bt * c0) * 8],
                        num_idxs=n0,
                        num_idxs_reg=n_regs[n0],
                        elem_size=128,
                        single_packet=False,
                        queue_num=(qoff + 2 * bi) % 4,
                    )
                    nc.gpsimd.dma_gather(
                        gb[:, bt * c0:nch, :],
                        table[half:n, :],
                        eidx_sb[:, (g_base + bt * c0) * 8:(g_base + nch) * 8],
                        num_idxs=n1,
                        num_idxs_reg=n_regs[n1],
                        elem_size=128,
                        single_packet=False,
                        queue_num=(qoff + 2 * bi + 1) % 4,
                    )
                    nc.sync.dma_start(stage[:, g_base:g_base + nch, :], gb[:])
                    g_base += nch
                # phase B: compute streams chunks back from the DRAM stage.
                g_base = 0
                for bi, batch in enumerate(cfg.batches):
                    bt = len(batch)
                    nch = bt * (c0 + c1)
                    lb = lp.tile([128, nch, 128], dt.float16, tag="lb")
                    nc.sync.dma_start(lb[:], stage[:, g_base:g_base + nch, :])
                    gb = lb
                    for i, t in enumerate(batch):
                        agg_ps = psA.tile([f_in, 128], dt.float32, tag="psA")
                        slots = list(range(i * c0, (i + 1) * c0)) + list(
                            range(bt * c0 + i * c1, bt * c0 + (i + 1) * c1)
                        )
                        for j, s in enumerate(slots):
                            g = g_base + s
                            V = vp.tile([128, 128], dt.float16, tag="v")
                            nc.vector.tensor_scalar(
                                V[:],
                                iota_sb[:],
                                edstl_sb[:, g:g + 1],
                                enorm_sb[:, g:g + 1],
                                ALU.is_equal,
                                ALU.mult,
                            )
                            nc.tensor.matmul(
                                agg_ps[:],
                                gb[:, s, 0:f_in],
                                V[:],
                                start=(j == 0),
                                stop=(j == len(slots) - 1),
                            )
                        agg_sb = sp.tile([f_in, 128], dt.float16, tag="agg")
                        nc.scalar.copy(agg_sb[:], agg_ps[:])
                        z_ps = psZ.tile([f_out, 128], dt.float32, tag="psZ")
                        nc.tensor.matmul(z_ps[:], w_sb[:], agg_sb[:], start=True, stop=True)
                        zr_sb = sp.tile([f_out, 128], out_dt, tag="zr")
                        nc.scalar.activation(zr_sb[:], z_ps[:], AF.Relu, bias=b_sb[:], scale=1.0)
                        tr_ps = psT.tile([128, f_out], out_dt, tag="psT")
                        nc.tensor.transpose(tr_ps[:], zr_sb[:], id_sb[:])
                        h_sb = sp.tile([128, f_out], out_dt, tag="h")
                        nc.scalar.copy(h_sb[:], tr_ps[:])
                        rows = cfg.last_rows if t == nt - 1 else 128
                        write_out(t, h_sb, rows)
                    g_base += nch

            def w1_out(t, h_sb, rows):
                nc.sync.dma_start(cc_in[t * 128:t * 128 + rows, :], h_sb[0:rows, :])

            do_layer(xt.ap(), F0, w1_sb, b1_sb, dt.float16, idA_sb, F1, w1_out, 0,
                     stage1)

            nc.gpsimd.collective_compute(
                "AllGather",
                ALU.bypass,
                replica_groups=[list(range(cfg.n_cores))],
                ins=[cc_in.opt()],
                outs=[cc_out.opt()],
            )

            def w2_out(t, h_sb, rows):
                nc.sync.dma_start(outt.ap()[t * 128:t * 128 + rows, :], h_sb[0:rows, :])

            do_layer(cc_out[:, :], F1, w2_sb, b2_sb, dt.float32, idB_sb, F2, w2_out, 2,
                     stage2)

    nc.compile()
    return nc


# ---------------------------------------------------------------------------
# Entry point
# ---------------------------------------------------------------------------

_CACHE = {}


def kernel(x, edge_index, W1, b1, W2, b2):
    x = np.asarray(x)
    cfg, in_maps = prepare(x, edge_index, W1, b1, W2, b2)

    key = (cfg.n, cfg.n_cores, cfg.c0, cfg.c1)
    nc = _CACHE.get(key)
    if nc is None:
        nc = build_program(cfg)
        _CACHE[key] = nc

    from concourse.bass_utils import run_bass_kernel_spmd

    res = run_bass_kernel_spmd(nc, in_maps, core_ids=list(range(cfg.n_cores)))
    out = np.concatenate([r["out"] for r in res.results], axis=0)
    return np.ascontiguousarray(out.astype(np.float32))


# revision 7
# speedup vs baseline: 1.5975x; 1.0217x over previous
"""Two-layer GCN (PyG GCNConv x2 + ReLU) on 8 Trainium2 NeuronCores.

Math: out = relu(S @ (relu(S @ (x W1) + b1) W2) + b2), with
S = D^-1/2 (A + I) D^-1/2 the symmetric-normalized adjacency (1.6M random
edges + self loops over 50000 nodes).

Key reformulation: aggregation is linear, so S (x W) == (S x) W.  Both
layers aggregate in the *small* (64-wide) feature space:
  layer1: agg = S x            (64 wide), h1 = relu(agg @ W1 + b1)
  layer2: agg = S (h1 W2)?  -> actually h1 is 128 wide; we aggregate h1
          directly (128 wide) and transform after: relu((S h1) @ W2 + b2).

Sharding: destination nodes are range-partitioned across the 8 cores
(6250 per core).  Each core owns the edges whose dst falls in its range
(plus its self-loops), pre-sorted by dst tile on the host.  Layer-1's
gather table is the (replicated) input x itself; layer-2's table is the
AllGather of the per-core h1 shards.  Weights are replicated.

Per 128-edge chunk the device does:
  - dma_gather: 128 rows of the node table -> SBUF [128 edges, 128 feats]
  - DVE tensor_scalar builds V[e,d] = (iota[d]==dstl[e]) * norm[e]
    (norm = dis[src]*dis[dst] precomputed per edge on the host; self
    loops are ordinary edges with norm = dis^2)
  - PE matmul accumulates agg.T[f,d] += gathered.T @ V in PSUM
Per 128-dst tile: transform with W (PE), bias+relu (ACT, bias is
per-partition in the transposed layout), transpose back to node-major
(PE), and DMA out.
"""

import math

import numpy as np

# ---------------------------------------------------------------------------
# Configuration
# ---------------------------------------------------------------------------


class Cfg:
    def __init__(self, n, n_cores, half, batch_tiles, c0, c1):
        self.n = n                       # total nodes
        self.n_cores = n_cores
        self.npc = n // n_cores          # nodes (dsts) per core
        self.nt = (self.npc + 127) // 128  # dst tiles per core
        self.last_rows = self.npc - (self.nt - 1) * 128
        self.half = half                 # table split point (int16 idx range)
        self.c0 = c0                     # chunks per tile, src < half
        self.c1 = c1                     # chunks per tile, src >= half
        self.nchunk = self.nt * (c0 + c1)
        self.batches = []
        t = 0
        while t < self.nt:
            self.batches.append(list(range(t, min(t + batch_tiles, self.nt))))
            t += batch_tiles
        self.f0, self.f1, self.f2 = 64, 128, 64


# ---------------------------------------------------------------------------
# Host-side preprocessing (graph partitioning / normalization structure)
# ---------------------------------------------------------------------------


def prepare(x, edge_index, W1, b1, W2, b2, n_cores=8, half=32768, batch_tiles=1):
    x = np.asarray(x, dtype=np.float32)
    edge_index = np.asarray(edge_index)
    W1 = np.asarray(W1, dtype=np.float32)
    b1 = np.asarray(b1, dtype=np.float32)
    W2 = np.asarray(W2, dtype=np.float32)
    b2 = np.asarray(b2, dtype=np.float32)

    n = x.shape[0]
    src = edge_index[0].astype(np.int64)
    dst = edge_index[1].astype(np.int64)

    deg = 1.0 + np.bincount(dst, minlength=n).astype(np.float64)
    dis = (1.0 / np.sqrt(deg)).astype(np.float32)

    # self loops as ordinary edges
    loops = np.arange(n, dtype=np.int64)
    src_all = np.concatenate([src, loops])
    dst_all = np.concatenate([dst, loops])
    norm_all = dis[src_all] * dis[dst_all]

    npc = n // n_cores
    core_of = dst_all // npc

    # pass 1: per-(core, tile, half) counts -> global uniform chunk counts
    per_core = []
    c0_max = 1
    c1_max = 1
    nt = (npc + 127) // 128
    for c in range(n_cores):
        m = core_of == c
        s = src_all[m]
        d = dst_all[m] - c * npc
        nr = norm_all[m]
        tile_id = d >> 7
        h = (s >= half).astype(np.int64)
        key = tile_id * 2 + h
        order = np.argsort(key, kind="stable")
        sk = key[order]
        bounds = np.searchsorted(sk, np.arange(nt * 2 + 1))
        cnt = np.diff(bounds)
        c0_max = max(c0_max, int(math.ceil(cnt[0::2].max() / 128.0)))
        c1_max = max(c1_max, int(math.ceil(cnt[1::2].max() / 128.0)))
        per_core.append((s, d, nr, order, bounds))

    cfg = Cfg(n, n_cores, half, batch_tiles, c0_max, c1_max)

    # pass 2: build padded streams in batch order
    maps = []
    for c in range(n_cores):
        s, d, nr, order, bounds = per_core[c]
        idx_stream = np.zeros(cfg.nchunk * 128, np.int16)
        dstl_stream = np.full(cfg.nchunk * 128, 500.0, np.float32)
        norm_stream = np.zeros(cfg.nchunk * 128, np.float32)
        pos = 0  # in chunks
        for batch in cfg.batches:
            for hh, cmax in ((0, cfg.c0), (1, cfg.c1)):
                for t in batch:
                    seg = order[bounds[2 * t + hh]:bounds[2 * t + hh + 1]]
                    L = len(seg)
                    base = pos * 128
                    sv = s[seg]
                    if hh:
                        sv = sv - half
                    idx_stream[base:base + L] = sv.astype(np.int16)
                    dstl_stream[base:base + L] = (d[seg] & 127).astype(np.float32)
                    norm_stream[base:base + L] = nr[seg]
                    pos += cmax
        assert pos == cfg.nchunk
        eidx = np.tile(idx_stream.reshape(cfg.nchunk * 8, 16).T, (8, 1))
        edstl = np.ascontiguousarray(dstl_stream.reshape(cfg.nchunk, 128).T)
        enorm = np.ascontiguousarray(norm_stream.reshape(cfg.nchunk, 128).T)
        maps.append({"eidx": eidx, "edstl": edstl, "enorm": enorm})

    xt = np.zeros((n, 128), np.float16)
    xt[:, :64] = x
    shared = {
        "xt": xt,
        "w1": W1.astype(np.float16),
        "b1": np.ascontiguousarray(b1.reshape(cfg.f1, 1)),
        "w2": W2.astype(np.float16),
        "b2": np.ascontiguousarray(b2.reshape(cfg.f2, 1)),
        "iota": np.tile(np.arange(128, dtype=np.float16), (128, 1)),
        "idA": np.eye(128, dtype=np.float16),
        "idB": np.eye(cfg.f2, dtype=np.float32),
    }
    in_maps = [{**shared, **m} for m in maps]
    return cfg, in_maps


# ---------------------------------------------------------------------------
# Device program
# ---------------------------------------------------------------------------


def build_program(cfg):
    import concourse.bacc as bacc
    import concourse.mybir as mybir
    import concourse.tile as tile

    dt = mybir.dt
    AF = mybir.ActivationFunctionType
    ALU = mybir.AluOpType

    n, npc, nt, half = cfg.n, cfg.npc, cfg.nt, cfg.half
    c0, c1 = cfg.c0, cfg.c1
    nchunk = cfg.nchunk
    F0, F1, F2 = cfg.f0, cfg.f1, cfg.f2

    nc = bacc.Bacc(
        "TRN2",
        target_bir_lowering=False,
        debug=False,
        enable_asserts=False,
        num_devices=cfg.n_cores,
        num_swdge_queues=4,
    )

    xt = nc.dram_tensor("xt", [n, 128], dt.float16, kind="ExternalInput")
    eidx = nc.dram_tensor("eidx", [128, nchunk * 8], dt.int16, kind="ExternalInput")
    edstl = nc.dram_tensor("edstl", [128, nchunk], dt.float32, kind="ExternalInput")
    enorm = nc.dram_tensor("enorm", [128, nchunk], dt.float32, kind="ExternalInput")
    w1t = nc.dram_tensor("w1", [F0, F1], dt.float16, kind="ExternalInput")
    b1t = nc.dram_tensor("b1", [F1, 1], dt.float32, kind="ExternalInput")
    w2t = nc.dram_tensor("w2", [F1, F2], dt.float16, kind="ExternalInput")
    b2t = nc.dram_tensor("b2", [F2, 1], dt.float32, kind="ExternalInput")
    iotat = nc.dram_tensor("iota", [128, 128], dt.float16, kind="ExternalInput")
    idAt = nc.dram_tensor("idA", [128, 128], dt.float16, kind="ExternalInput")
    idBt = nc.dram_tensor("idB", [F2, F2], dt.float32, kind="ExternalInput")
    outt = nc.dram_tensor("out", [npc, F2], dt.float32, kind="ExternalOutput")

    with tile.TileContext(nc) as tc:
        with (
            tc.tile_pool(name="const", bufs=1) as cp,
            tc.tile_pool(name="edges", bufs=1) as ep,
            tc.tile_pool(name="gb", bufs=4) as gp,
            tc.tile_pool(name="lb", bufs=3) as lp,
            tc.tile_pool(name="v", bufs=12) as vp,
            tc.tile_pool(name="sb", bufs=3) as sp,
            tc.tile_pool(name="psA", bufs=2, space="PSUM") as psA,
            tc.tile_pool(name="psZ", bufs=2, space="PSUM") as psZ,
            tc.tile_pool(name="psT", bufs=2, space="PSUM") as psT,
            tc.tile_pool(name="dram", bufs=1, space="DRAM") as dp,
        ):
            def load_const(pool, t, dtype):
                sb = pool.tile(list(t.shape), dtype, tag=t.name)
                nc.sync.dma_start(sb[:], t.ap()[:])
                return sb

            iota_sb = load_const(cp, iotat, dt.float16)
            idA_sb = load_const(cp, idAt, dt.float16)
            idB_sb = load_const(cp, idBt, dt.float32)
            w1_sb = load_const(cp, w1t, dt.float16)
            b1_sb = load_const(cp, b1t, dt.float32)
            w2_sb = load_const(cp, w2t, dt.float16)
            b2_sb = load_const(cp, b2t, dt.float32)
            eidx_sb = load_const(ep, eidx, dt.int16)
            edstl_sb = load_const(ep, edstl, dt.float32)
            enorm_sb = load_const(ep, enorm, dt.float32)

            cc_in = dp.tile([npc, F1], dt.float16)
            cc_out = dp.tile([n, F1], dt.float16)
            stage1 = dp.tile([128, nchunk, 128], dt.float16)
            stage2 = dp.tile([128, nchunk, 128], dt.float16)

            n_regs = {}
            for bt in {len(b) for b in cfg.batches}:
                n_regs[bt * c0 * 128] = nc.gpsimd.to_reg(bt * c0 * 128)
                n_regs[bt * c1 * 128] = nc.gpsimd.to_reg(bt * c1 * 128)

            def do_layer(table, f_in, w_sb, b_sb, out_dt, id_sb, f_out, write_out,
                         qoff, stage):
                # phase A: gathers stream into SBUF and are immediately staged
                # out to DRAM; slots recycle fast so desc-gen pipelines across
                # the 4 SWDGE queue core-pairs without waiting on compute.
                g_base = 0
                for bi, batch in enumerate(cfg.batches):
                    bt = len(batch)
                    nch = bt * (c0 + c1)
                    gb = gp.tile([128, nch, 128], dt.float16, tag="gb")
                    n0 = bt * c0 * 128
                    n1 = bt * c1 * 128
                    nc.gpsimd.dma_gather(
                        gb[:, 0:bt * c0, :],
                        table[0:half, :],
                        eidx_sb[:, g_base * 8:(g_base + bt * c0) * 8],
                        num_idxs=n0,
                        num_idxs_reg=n_regs[n0],
                        elem_size=128,
                        single_packet=False,
                        queue_num=(qoff + 2 * bi) % 4,
                    )
                    nc.gpsimd.dma_gather(
                        gb[:, bt * c0:nch, :],
                        table[half:n, :],
                        eidx_sb[:, (g_base + bt * c0) * 8:(g_base + nch) * 8],
                        num_idxs=n1,
                        num_idxs_reg=n_regs[n1],
                        elem_size=128,
                        single_packet=False,
                        queue_num=(qoff + 2 * bi + 1) % 4,
                    )
                    nc.sync.dma_start(stage[:, g_base:g_base + nch, :], gb[:])
                    g_base += nch
                # phase B: compute streams chunks back from the DRAM stage.
                g_base = 0
                for bi, batch in enumerate(cfg.batches):
                    bt = len(batch)
                    nch = bt * (c0 + c1)
                    lb = lp.tile([128, nch, 128], dt.float16, tag="lb")
                    nc.scalar.dma_start(lb[:], stage[:, g_base:g_base + nch, :])
                    gb = lb
                    for i, t in enumerate(batch):
                        agg_ps = psA.tile([f_in, 128], dt.float32, tag="psA")
                        slots = list(range(i * c0, (i + 1) * c0)) + list(
                            range(bt * c0 + i * c1, bt * c0 + (i + 1) * c1)
                        )
                        for j, s in enumerate(slots):
                            g = g_base + s
                            V = vp.tile([128, 128], dt.float16, tag="v")
                            nc.vector.tensor_scalar(
                                V[:],
                                iota_sb[:],
                                edstl_sb[:, g:g + 1],
                                enorm_sb[:, g:g + 1],
                                ALU.is_equal,
                                ALU.mult,
                            )
                            nc.tensor.matmul(
                                agg_ps[:],
                                gb[:, s, 0:f_in],
                                V[:],
                                start=(j == 0),
                                stop=(j == len(slots) - 1),
                            )
                        agg_sb = sp.tile([f_in, 128], dt.float16, tag="agg")
                        nc.scalar.copy(agg_sb[:], agg_ps[:])
                        z_ps = psZ.tile([f_out, 128], dt.float32, tag="psZ")
                        nc.tensor.matmul(z_ps[:], w_sb[:], agg_sb[:], start=True, stop=True)
                        zr_sb = sp.tile([f_out, 128], out_dt, tag="zr")
                        nc.scalar.activation(zr_sb[:], z_ps[:], AF.Relu, bias=b_sb[:], scale=1.0)
                        tr_ps = psT.tile([128, f_out], out_dt, tag="psT")
                        nc.tensor.transpose(tr_ps[:], zr_sb[:], id_sb[:])
                        h_sb = sp.tile([128, f_out], out_dt, tag="h")
                        nc.scalar.copy(h_sb[:], tr_ps[:])
                        rows = cfg.last_rows if t == nt - 1 else 128
                        write_out(t, h_sb, rows)
                    g_base += nch

            def w1_out(t, h_sb, rows):
                nc.sync.dma_start(cc_in[t * 128:t * 128 + rows, :], h_sb[0:rows, :])

            do_layer(xt.ap(), F0, w1_sb, b1_sb, dt.float16, idA_sb, F1, w1_out, 0,
                     stage1)

            nc.gpsimd.collective_compute(
                "AllGather",
                ALU.bypass,
                replica_groups=[list(range(cfg.n_cores))],
                ins=[cc_in.opt()],
                outs=[cc_out.opt()],
            )

            def w2_out(t, h_sb, rows):
                nc.sync.dma_start(outt.ap()[t * 128:t * 128 + rows, :], h_sb[0:rows, :])

            do_layer(cc_out[:, :], F1, w2_sb, b2_sb, dt.float32, idB_sb, F2, w2_out, 2,
                     stage2)

    nc.compile()
    return nc


# ---------------------------------------------------------------------------
# Entry point
# ---------------------------------------------------------------------------

_CACHE = {}


def kernel(x, edge_index, W1, b1, W2, b2):
    x = np.asarray(x)
    cfg, in_maps = prepare(x, edge_index, W1, b1, W2, b2)

    key = (cfg.n, cfg.n_cores, cfg.c0, cfg.c1)
    nc = _CACHE.get(key)
    if nc is None:
        nc = build_program(cfg)
        _CACHE[key] = nc

    from concourse.bass_utils import run_bass_kernel_spmd

    res = run_bass_kernel_spmd(nc, in_maps, core_ids=list(range(cfg.n_cores)))
    out = np.concatenate([r["out"] for r in res.results], axis=0)
    return np.ascontiguousarray(out.astype(np.float32))


# revision 8
# speedup vs baseline: 1.7169x; 1.0747x over previous
"""Two-layer GCN (PyG GCNConv x2 + ReLU) on 8 Trainium2 NeuronCores.

Math: out = relu(S @ (relu(S @ (x W1) + b1) W2) + b2), with
S = D^-1/2 (A + I) D^-1/2 the symmetric-normalized adjacency (1.6M random
edges + self loops over 50000 nodes).

Key reformulation: aggregation is linear, so S (x W) == (S x) W.  Both
layers aggregate in the *small* (64-wide) feature space:
  layer1: agg = S x            (64 wide), h1 = relu(agg @ W1 + b1)
  layer2: agg = S (h1 W2)?  -> actually h1 is 128 wide; we aggregate h1
          directly (128 wide) and transform after: relu((S h1) @ W2 + b2).

Sharding: destination nodes are range-partitioned across the 8 cores
(6250 per core).  Each core owns the edges whose dst falls in its range
(plus its self-loops), pre-sorted by dst tile on the host.  Layer-1's
gather table is the (replicated) input x itself; layer-2's table is the
AllGather of the per-core h1 shards.  Weights are replicated.

Per 128-edge chunk the device does:
  - dma_gather: 128 rows of the node table -> SBUF [128 edges, 128 feats]
  - DVE tensor_scalar builds V[e,d] = (iota[d]==dstl[e]) * norm[e]
    (norm = dis[src]*dis[dst] precomputed per edge on the host; self
    loops are ordinary edges with norm = dis^2)
  - PE matmul accumulates agg.T[f,d] += gathered.T @ V in PSUM
Per 128-dst tile: transform with W (PE), bias+relu (ACT, bias is
per-partition in the transposed layout), transpose back to node-major
(PE), and DMA out.
"""

import math

import numpy as np

# ---------------------------------------------------------------------------
# Configuration
# ---------------------------------------------------------------------------


class Cfg:
    def __init__(self, n, n_cores, half, batch_tiles, c0, c1):
        self.n = n                       # total nodes
        self.n_cores = n_cores
        self.npc = n // n_cores          # nodes (dsts) per core
        self.nt = (self.npc + 127) // 128  # dst tiles per core
        self.last_rows = self.npc - (self.nt - 1) * 128
        self.half = half                 # table split point (int16 idx range)
        self.c0 = c0                     # chunks per tile, src < half
        self.c1 = c1                     # chunks per tile, src >= half
        self.nchunk = self.nt * (c0 + c1)
        self.batches = []
        t = 0
        while t < self.nt:
            self.batches.append(list(range(t, min(t + batch_tiles, self.nt))))
            t += batch_tiles
        self.f0, self.f1, self.f2 = 64, 128, 64


# ---------------------------------------------------------------------------
# Host-side preprocessing (graph partitioning / normalization structure)
# ---------------------------------------------------------------------------


def prepare(x, edge_index, W1, b1, W2, b2, n_cores=8, half=32768, batch_tiles=1):
    x = np.asarray(x, dtype=np.float32)
    edge_index = np.asarray(edge_index)
    W1 = np.asarray(W1, dtype=np.float32)
    b1 = np.asarray(b1, dtype=np.float32)
    W2 = np.asarray(W2, dtype=np.float32)
    b2 = np.asarray(b2, dtype=np.float32)

    n = x.shape[0]
    src = edge_index[0].astype(np.int64)
    dst = edge_index[1].astype(np.int64)

    deg = 1.0 + np.bincount(dst, minlength=n).astype(np.float64)
    dis = (1.0 / np.sqrt(deg)).astype(np.float32)

    # self loops as ordinary edges
    loops = np.arange(n, dtype=np.int64)
    src_all = np.concatenate([src, loops])
    dst_all = np.concatenate([dst, loops])
    norm_all = dis[src_all] * dis[dst_all]

    npc = n // n_cores
    core_of = dst_all // npc

    # pass 1: per-(core, tile, half) counts -> global uniform chunk counts
    per_core = []
    c0_max = 1
    c1_max = 1
    nt = (npc + 127) // 128
    for c in range(n_cores):
        m = core_of == c
        s = src_all[m]
        d = dst_all[m] - c * npc
        nr = norm_all[m]
        tile_id = d >> 7
        h = (s >= half).astype(np.int64)
        key = tile_id * 2 + h
        order = np.argsort(key, kind="stable")
        sk = key[order]
        bounds = np.searchsorted(sk, np.arange(nt * 2 + 1))
        cnt = np.diff(bounds)
        c0_max = max(c0_max, int(math.ceil(cnt[0::2].max() / 128.0)))
        c1_max = max(c1_max, int(math.ceil(cnt[1::2].max() / 128.0)))
        per_core.append((s, d, nr, order, bounds))

    cfg = Cfg(n, n_cores, half, batch_tiles, c0_max, c1_max)

    # pass 2: build padded streams in batch order
    maps = []
    for c in range(n_cores):
        s, d, nr, order, bounds = per_core[c]
        idx_stream = np.zeros(cfg.nchunk * 128, np.int16)
        dstl_stream = np.full(cfg.nchunk * 128, 500.0, np.float32)
        norm_stream = np.zeros(cfg.nchunk * 128, np.float32)
        pos = 0  # in chunks
        for batch in cfg.batches:
            for hh, cmax in ((0, cfg.c0), (1, cfg.c1)):
                for t in batch:
                    seg = order[bounds[2 * t + hh]:bounds[2 * t + hh + 1]]
                    L = len(seg)
                    base = pos * 128
                    sv = s[seg]
                    if hh:
                        sv = sv - half
                    idx_stream[base:base + L] = sv.astype(np.int16)
                    dstl_stream[base:base + L] = (d[seg] & 127).astype(np.float32)
                    norm_stream[base:base + L] = nr[seg]
                    pos += cmax
        assert pos == cfg.nchunk
        eidx = np.tile(idx_stream.reshape(cfg.nchunk * 8, 16).T, (8, 1))
        edstl = np.ascontiguousarray(dstl_stream.reshape(cfg.nchunk, 128).T)
        enorm = np.ascontiguousarray(norm_stream.reshape(cfg.nchunk, 128).T)
        maps.append({"eidx": eidx, "edstl": edstl, "enorm": enorm})

    xt = np.zeros((n, 128), np.float16)
    xt[:, :64] = x
    shared = {
        "xt": xt,
        "w1": W1.astype(np.float16),
        "b1": np.ascontiguousarray(b1.reshape(cfg.f1, 1)),
        "w2": W2.astype(np.float16),
        "b2": np.ascontiguousarray(b2.reshape(cfg.f2, 1)),
        "iota": np.tile(np.arange(128, dtype=np.float16), (128, 1)),
        "idA": np.eye(128, dtype=np.float16),
        "idB": np.eye(cfg.f2, dtype=np.float32),
    }
    in_maps = [{**shared, **m} for m in maps]
    return cfg, in_maps


# ---------------------------------------------------------------------------
# Device program
# ---------------------------------------------------------------------------


def build_program(cfg):
    import concourse.bacc as bacc
    import concourse.mybir as mybir
    import concourse.tile as tile

    dt = mybir.dt
    AF = mybir.ActivationFunctionType
    ALU = mybir.AluOpType

    n, npc, nt, half = cfg.n, cfg.npc, cfg.nt, cfg.half
    c0, c1 = cfg.c0, cfg.c1
    nchunk = cfg.nchunk
    F0, F1, F2 = cfg.f0, cfg.f1, cfg.f2

    nc = bacc.Bacc(
        "TRN2",
        target_bir_lowering=False,
        debug=False,
        enable_asserts=False,
        num_devices=cfg.n_cores,
        num_swdge_queues=4,
    )

    xt = nc.dram_tensor("xt", [n, 128], dt.float16, kind="ExternalInput")
    eidx = nc.dram_tensor("eidx", [128, nchunk * 8], dt.int16, kind="ExternalInput")
    edstl = nc.dram_tensor("edstl", [128, nchunk], dt.float32, kind="ExternalInput")
    enorm = nc.dram_tensor("enorm", [128, nchunk], dt.float32, kind="ExternalInput")
    w1t = nc.dram_tensor("w1", [F0, F1], dt.float16, kind="ExternalInput")
    b1t = nc.dram_tensor("b1", [F1, 1], dt.float32, kind="ExternalInput")
    w2t = nc.dram_tensor("w2", [F1, F2], dt.float16, kind="ExternalInput")
    b2t = nc.dram_tensor("b2", [F2, 1], dt.float32, kind="ExternalInput")
    iotat = nc.dram_tensor("iota", [128, 128], dt.float16, kind="ExternalInput")
    idAt = nc.dram_tensor("idA", [128, 128], dt.float16, kind="ExternalInput")
    idBt = nc.dram_tensor("idB", [F2, F2], dt.float32, kind="ExternalInput")
    outt = nc.dram_tensor("out", [npc, F2], dt.float32, kind="ExternalOutput")

    with tile.TileContext(nc) as tc:
        with (
            tc.tile_pool(name="const", bufs=1) as cp,
            tc.tile_pool(name="edges", bufs=1) as ep,
            tc.tile_pool(name="gb", bufs=6) as gp,
            tc.tile_pool(name="lb", bufs=4) as lp,
            tc.tile_pool(name="v", bufs=12) as vp,
            tc.tile_pool(name="sb", bufs=3) as sp,
            tc.tile_pool(name="psA", bufs=3, space="PSUM") as psA,
            tc.tile_pool(name="psZ", bufs=2, space="PSUM") as psZ,
            tc.tile_pool(name="psT", bufs=2, space="PSUM") as psT,
            tc.tile_pool(name="dram", bufs=1, space="DRAM") as dp,
        ):
            def load_const(pool, t, dtype):
                sb = pool.tile(list(t.shape), dtype, tag=t.name)
                nc.sync.dma_start(sb[:], t.ap()[:])
                return sb

            iota_sb = load_const(cp, iotat, dt.float16)
            idA_sb = load_const(cp, idAt, dt.float16)
            idB_sb = load_const(cp, idBt, dt.float32)
            w1_sb = load_const(cp, w1t, dt.float16)
            b1_sb = load_const(cp, b1t, dt.float32)
            w2_sb = load_const(cp, w2t, dt.float16)
            b2_sb = load_const(cp, b2t, dt.float32)
            eidx_sb = load_const(ep, eidx, dt.int16)
            edstl_sb = load_const(ep, edstl, dt.float32)
            enorm_sb = load_const(ep, enorm, dt.float32)

            cc_in = dp.tile([npc, F1], dt.float16)
            cc_out = dp.tile([n, F1], dt.float16)
            stage1 = dp.tile([128, nchunk, 128], dt.float16)
            stage2 = dp.tile([128, nchunk, 128], dt.float16)

            n_regs = {}
            for bt in {len(b) for b in cfg.batches}:
                n_regs[bt * c0 * 128] = nc.gpsimd.to_reg(bt * c0 * 128)
                n_regs[bt * c1 * 128] = nc.gpsimd.to_reg(bt * c1 * 128)

            def do_layer(table, f_in, w_sb, b_sb, out_dt, id_sb, f_out, write_out,
                         qoff, stage):
                # phase A: gathers stream into SBUF and are immediately staged
                # out to DRAM; slots recycle fast so desc-gen pipelines across
                # the 4 SWDGE queue core-pairs without waiting on compute.
                g_base = 0
                for bi, batch in enumerate(cfg.batches):
                    bt = len(batch)
                    nch = bt * (c0 + c1)
                    gb = gp.tile([128, nch, 128], dt.float16, tag="gb")
                    n0 = bt * c0 * 128
                    n1 = bt * c1 * 128
                    nc.gpsimd.dma_gather(
                        gb[:, 0:bt * c0, :],
                        table[0:half, :],
                        eidx_sb[:, g_base * 8:(g_base + bt * c0) * 8],
                        num_idxs=n0,
                        num_idxs_reg=n_regs[n0],
                        elem_size=128,
                        single_packet=False,
                        queue_num=(qoff + 2 * bi) % 4,
                    )
                    nc.gpsimd.dma_gather(
                        gb[:, bt * c0:nch, :],
                        table[half:n, :],
                        eidx_sb[:, (g_base + bt * c0) * 8:(g_base + nch) * 8],
                        num_idxs=n1,
                        num_idxs_reg=n_regs[n1],
                        elem_size=128,
                        single_packet=False,
                        queue_num=(qoff + 2 * bi + 1) % 4,
                    )
                    nc.sync.dma_start(stage[:, g_base:g_base + nch, :], gb[:])
                    g_base += nch
                # phase B: compute streams chunks back from the DRAM stage.
                g_base = 0
                for bi, batch in enumerate(cfg.batches):
                    bt = len(batch)
                    nch = bt * (c0 + c1)
                    lb = lp.tile([128, nch, 128], dt.float16, tag="lb")
                    nc.scalar.dma_start(lb[:], stage[:, g_base:g_base + nch, :])
                    gb = lb
                    for i, t in enumerate(batch):
                        agg_ps = psA.tile([f_in, 128], dt.float32, tag="psA")
                        slots = list(range(i * c0, (i + 1) * c0)) + list(
                            range(bt * c0 + i * c1, bt * c0 + (i + 1) * c1)
                        )
                        for j, s in enumerate(slots):
                            g = g_base + s
                            V = vp.tile([128, 128], dt.float16, tag="v")
                            nc.vector.tensor_scalar(
                                V[:],
                                iota_sb[:],
                                edstl_sb[:, g:g + 1],
                                enorm_sb[:, g:g + 1],
                                ALU.is_equal,
                                ALU.mult,
                            )
                            nc.tensor.matmul(
                                agg_ps[:],
                                gb[:, s, 0:f_in],
                                V[:],
                                start=(j == 0),
                                stop=(j == len(slots) - 1),
                            )
                        agg_sb = sp.tile([f_in, 128], dt.float16, tag="agg")
                        nc.scalar.copy(agg_sb[:], agg_ps[:])
                        z_ps = psZ.tile([f_out, 128], dt.float32, tag="psZ")
                        nc.tensor.matmul(z_ps[:], w_sb[:], agg_sb[:], start=True, stop=True)
                        zr_sb = sp.tile([f_out, 128], out_dt, tag="zr")
                        nc.scalar.activation(zr_sb[:], z_ps[:], AF.Relu, bias=b_sb[:], scale=1.0)
                        tr_ps = psT.tile([128, f_out], out_dt, tag="psT")
                        nc.tensor.transpose(tr_ps[:], zr_sb[:], id_sb[:])
                        h_sb = sp.tile([128, f_out], out_dt, tag="h")
                        nc.scalar.copy(h_sb[:], tr_ps[:])
                        rows = cfg.last_rows if t == nt - 1 else 128
                        write_out(t, h_sb, rows)
                    g_base += nch

            def w1_out(t, h_sb, rows):
                nc.sync.dma_start(cc_in[t * 128:t * 128 + rows, :], h_sb[0:rows, :])

            do_layer(xt.ap(), F0, w1_sb, b1_sb, dt.float16, idA_sb, F1, w1_out, 0,
                     stage1)

            nc.gpsimd.collective_compute(
                "AllGather",
                ALU.bypass,
                replica_groups=[list(range(cfg.n_cores))],
                ins=[cc_in.opt()],
                outs=[cc_out.opt()],
            )

            def w2_out(t, h_sb, rows):
                nc.sync.dma_start(outt.ap()[t * 128:t * 128 + rows, :], h_sb[0:rows, :])

            do_layer(cc_out[:, :], F1, w2_sb, b2_sb, dt.float32, idB_sb, F2, w2_out, 2,
                     stage2)

    nc.compile()
    return nc


# ---------------------------------------------------------------------------
# Entry point
# ---------------------------------------------------------------------------

_CACHE = {}


def kernel(x, edge_index, W1, b1, W2, b2):
    x = np.asarray(x)
    cfg, in_maps = prepare(x, edge_index, W1, b1, W2, b2)

    key = (cfg.n, cfg.n_cores, cfg.c0, cfg.c1)
    nc = _CACHE.get(key)
    if nc is None:
        nc = build_program(cfg)
        _CACHE[key] = nc

    from concourse.bass_utils import run_bass_kernel_spmd

    res = run_bass_kernel_spmd(nc, in_maps, core_ids=list(range(cfg.n_cores)))
    out = np.concatenate([r["out"] for r in res.results], axis=0)
    return np.ascontiguousarray(out.astype(np.float32))


# revision 10
# speedup vs baseline: 2.4071x; 1.4020x over previous
"""Two-layer GCN (PyG GCNConv x2 + ReLU) on 8 Trainium2 NeuronCores.

Math: out = relu(S @ (relu(S @ (x W1) + b1) W2) + b2), with
S = D^-1/2 (A + I) D^-1/2 the symmetric-normalized adjacency (1.6M random
edges + self loops over 50000 nodes).

Key reformulation: aggregation is linear, so S (x W) == (S x) W.  Both
layers aggregate in the *small* (64-wide) feature space:
  layer1: agg = S x            (64 wide), h1 = relu(agg @ W1 + b1)
  layer2: agg = S (h1 W2)?  -> actually h1 is 128 wide; we aggregate h1
          directly (128 wide) and transform after: relu((S h1) @ W2 + b2).

Sharding: destination nodes are range-partitioned across the 8 cores
(6250 per core).  Each core owns the edges whose dst falls in its range
(plus its self-loops), pre-sorted by dst tile on the host.  Layer-1's
gather table is the (replicated) input x itself; layer-2's table is the
AllGather of the per-core h1 shards.  Weights are replicated.

Per 128-edge chunk the device does:
  - dma_gather: 128 rows of the node table -> SBUF [128 edges, 128 feats]
  - DVE tensor_scalar builds V[e,d] = (iota[d]==dstl[e]) * norm[e]
    (norm = dis[src]*dis[dst] precomputed per edge on the host; self
    loops are ordinary edges with norm = dis^2)
  - PE matmul accumulates agg.T[f,d] += gathered.T @ V in PSUM
Per 128-dst tile: transform with W (PE), bias+relu (ACT, bias is
per-partition in the transposed layout), transpose back to node-major
(PE), and DMA out.
"""

import math

import numpy as np

# ---------------------------------------------------------------------------
# Configuration
# ---------------------------------------------------------------------------


class Cfg:
    def __init__(self, n, n_cores, half, batch_tiles, c0, c1):
        self.n = n                       # total nodes
        self.n_cores = n_cores
        self.npc = n // n_cores          # nodes (dsts) per core
        self.nt = (self.npc + 127) // 128  # dst tiles per core
        self.last_rows = self.npc - (self.nt - 1) * 128
        self.half = half                 # table split point (int16 idx range)
        self.c0 = c0                     # chunks per tile, src < half
        self.c1 = c1                     # chunks per tile, src >= half
        self.nchunk = self.nt * (c0 + c1)
        self.batches = []
        t = 0
        while t < self.nt:
            self.batches.append(list(range(t, min(t + batch_tiles, self.nt))))
            t += batch_tiles
        self.f0, self.f1, self.f2 = 64, 128, 64


# ---------------------------------------------------------------------------
# Host-side preprocessing (graph partitioning / normalization structure)
# ---------------------------------------------------------------------------


def prepare(x, edge_index, W1, b1, W2, b2, n_cores=8, half=32768, batch_tiles=1):
    x = np.asarray(x, dtype=np.float32)
    edge_index = np.asarray(edge_index)
    W1 = np.asarray(W1, dtype=np.float32)
    b1 = np.asarray(b1, dtype=np.float32)
    W2 = np.asarray(W2, dtype=np.float32)
    b2 = np.asarray(b2, dtype=np.float32)

    n = x.shape[0]
    src = edge_index[0].astype(np.int64)
    dst = edge_index[1].astype(np.int64)

    deg = 1.0 + np.bincount(dst, minlength=n).astype(np.float64)
    dis = (1.0 / np.sqrt(deg)).astype(np.float32)

    # self loops as ordinary edges
    loops = np.arange(n, dtype=np.int64)
    src_all = np.concatenate([src, loops])
    dst_all = np.concatenate([dst, loops])
    norm_all = dis[src_all] * dis[dst_all]

    npc = n // n_cores
    core_of = dst_all // npc

    # pass 1: per-(core, tile, half) counts -> global uniform chunk counts
    per_core = []
    c0_max = 1
    c1_max = 1
    nt = (npc + 127) // 128
    for c in range(n_cores):
        m = core_of == c
        s = src_all[m]
        d = dst_all[m] - c * npc
        nr = norm_all[m]
        tile_id = d >> 7
        h = (s >= half).astype(np.int64)
        key = tile_id * 2 + h
        order = np.argsort(key, kind="stable")
        sk = key[order]
        bounds = np.searchsorted(sk, np.arange(nt * 2 + 1))
        cnt = np.diff(bounds)
        c0_max = max(c0_max, int(math.ceil(cnt[0::2].max() / 128.0)))
        c1_max = max(c1_max, int(math.ceil(cnt[1::2].max() / 128.0)))
        per_core.append((s, d, nr, order, bounds))

    cfg = Cfg(n, n_cores, half, batch_tiles, c0_max, c1_max)

    # pass 2: build padded streams in batch order
    maps = []
    for c in range(n_cores):
        s, d, nr, order, bounds = per_core[c]
        idx_stream = np.zeros(cfg.nchunk * 128, np.int16)
        dstl_stream = np.full(cfg.nchunk * 128, 500.0, np.float32)
        norm_stream = np.zeros(cfg.nchunk * 128, np.float32)
        pos = 0  # in chunks
        for batch in cfg.batches:
            for hh, cmax in ((0, cfg.c0), (1, cfg.c1)):
                for t in batch:
                    seg = order[bounds[2 * t + hh]:bounds[2 * t + hh + 1]]
                    L = len(seg)
                    base = pos * 128
                    sv = s[seg]
                    if hh:
                        sv = sv - half
                    idx_stream[base:base + L] = sv.astype(np.int16)
                    dstl_stream[base:base + L] = (d[seg] & 127).astype(np.float32)
                    norm_stream[base:base + L] = nr[seg]
                    pos += cmax
        assert pos == cfg.nchunk
        eidx = np.tile(idx_stream.reshape(cfg.nchunk * 8, 16).T, (8, 1))
        edstl = np.ascontiguousarray(dstl_stream.reshape(cfg.nchunk, 128).T)
        enorm = np.ascontiguousarray(norm_stream.reshape(cfg.nchunk, 128).T)
        maps.append({"eidx": eidx, "edstl": edstl, "enorm": enorm})

    xt = np.zeros((n, 128), np.float16)
    xt[:, :64] = x
    shared = {
        "xt": xt,
        "w1": W1.astype(np.float16),
        "b1": np.ascontiguousarray(b1.reshape(cfg.f1, 1)),
        "w2": W2.astype(np.float16),
        "b2": np.ascontiguousarray(b2.reshape(cfg.f2, 1)),
        "iota": np.tile(np.arange(128, dtype=np.float16), (128, 1)),
        "idA": np.eye(128, dtype=np.float16),
        "idB": np.eye(cfg.f2, dtype=np.float32),
    }
    in_maps = [{**shared, **m} for m in maps]
    return cfg, in_maps


# ---------------------------------------------------------------------------
# Device program
# ---------------------------------------------------------------------------


def build_program(cfg):
    import concourse.bacc as bacc
    import concourse.mybir as mybir
    import concourse.tile as tile

    dt = mybir.dt
    AF = mybir.ActivationFunctionType
    ALU = mybir.AluOpType

    n, npc, nt, half = cfg.n, cfg.npc, cfg.nt, cfg.half
    c0, c1 = cfg.c0, cfg.c1
    nchunk = cfg.nchunk
    F0, F1, F2 = cfg.f0, cfg.f1, cfg.f2

    nc = bacc.Bacc(
        "TRN2",
        target_bir_lowering=False,
        debug=False,
        enable_asserts=False,
        num_devices=cfg.n_cores,
        num_swdge_queues=4,
    )

    xt = nc.dram_tensor("xt", [n, 128], dt.float16, kind="ExternalInput")
    eidx = nc.dram_tensor("eidx", [128, nchunk * 8], dt.int16, kind="ExternalInput")
    edstl = nc.dram_tensor("edstl", [128, nchunk], dt.float32, kind="ExternalInput")
    enorm = nc.dram_tensor("enorm", [128, nchunk], dt.float32, kind="ExternalInput")
    w1t = nc.dram_tensor("w1", [F0, F1], dt.float16, kind="ExternalInput")
    b1t = nc.dram_tensor("b1", [F1, 1], dt.float32, kind="ExternalInput")
    w2t = nc.dram_tensor("w2", [F1, F2], dt.float16, kind="ExternalInput")
    b2t = nc.dram_tensor("b2", [F2, 1], dt.float32, kind="ExternalInput")
    iotat = nc.dram_tensor("iota", [128, 128], dt.float16, kind="ExternalInput")
    idAt = nc.dram_tensor("idA", [128, 128], dt.float16, kind="ExternalInput")
    idBt = nc.dram_tensor("idB", [F2, F2], dt.float32, kind="ExternalInput")
    outt = nc.dram_tensor("out", [npc, F2], dt.float32, kind="ExternalOutput")

    with tile.TileContext(nc) as tc:
        with (
            tc.tile_pool(name="const", bufs=1) as cp,
            tc.tile_pool(name="edges", bufs=1) as ep,
            tc.tile_pool(name="gb", bufs=6) as gp,
            tc.tile_pool(name="lb", bufs=4) as lp,
            tc.tile_pool(name="v", bufs=12) as vp,
            tc.tile_pool(name="sb", bufs=3) as sp,
            tc.tile_pool(name="psA", bufs=3, space="PSUM") as psA,
            tc.tile_pool(name="psZ", bufs=2, space="PSUM") as psZ,
            tc.tile_pool(name="psT", bufs=2, space="PSUM") as psT,
            tc.tile_pool(name="dram", bufs=1, space="DRAM") as dp,
        ):
            def load_const(pool, t, dtype):
                sb = pool.tile(list(t.shape), dtype, tag=t.name)
                nc.sync.dma_start(sb[:], t.ap()[:])
                return sb

            iota_sb = load_const(cp, iotat, dt.float16)
            idA_sb = load_const(cp, idAt, dt.float16)
            idB_sb = load_const(cp, idBt, dt.float32)
            w1_sb = load_const(cp, w1t, dt.float16)
            b1_sb = load_const(cp, b1t, dt.float32)
            w2_sb = load_const(cp, w2t, dt.float16)
            b2_sb = load_const(cp, b2t, dt.float32)
            eidx_sb = load_const(ep, eidx, dt.int16)
            edstl_sb = load_const(ep, edstl, dt.float32)
            enorm_sb = load_const(ep, enorm, dt.float32)

            cc_in = dp.tile([npc, F1], dt.float16)
            cc_out = dp.tile([n, F1], dt.float16)
            stage1 = dp.tile([128, nchunk, 128], dt.float16)
            stage2 = dp.tile([128, nchunk, 128], dt.float16)

            n_regs = {}
            for bt in {len(b) for b in cfg.batches}:
                n_regs[bt * c0 * 128] = nc.gpsimd.to_reg(bt * c0 * 128)
                n_regs[bt * c1 * 128] = nc.gpsimd.to_reg(bt * c1 * 128)

            def do_layer(table, f_in, w_sb, b_sb, out_dt, id_sb, f_out, write_out,
                         qoff, stage):
                # phase A: gathers stream into SBUF and are immediately staged
                # out to DRAM; slots recycle fast so desc-gen pipelines across
                # the 4 SWDGE queue core-pairs without waiting on compute.
                g_base = 0
                for bi, batch in enumerate(cfg.batches):
                    bt = len(batch)
                    nch = bt * (c0 + c1)
                    gb = gp.tile([128, nch, 128], dt.float16, tag="gb")
                    n0 = bt * c0 * 128
                    n1 = bt * c1 * 128
                    nc.gpsimd.dma_gather(
                        gb[:, 0:bt * c0, :],
                        table[0:half, :],
                        eidx_sb[:, g_base * 8:(g_base + bt * c0) * 8],
                        num_idxs=n0,
                        num_idxs_reg=n_regs[n0],
                        elem_size=128,
                        single_packet=False,
                        queue_num=(qoff + 2 * bi) % 4,
                    )
                    nc.gpsimd.dma_gather(
                        gb[:, bt * c0:nch, :],
                        table[half:n, :],
                        eidx_sb[:, (g_base + bt * c0) * 8:(g_base + nch) * 8],
                        num_idxs=n1,
                        num_idxs_reg=n_regs[n1],
                        elem_size=128,
                        single_packet=False,
                        queue_num=(qoff + 2 * bi + 1) % 4,
                    )
                    nc.sync.dma_start(stage[:, g_base:g_base + nch, :], gb[:])
                    g_base += nch
                # phase B: compute streams chunks back from the DRAM stage.
                import os
                if os.environ.get("K_SKIP_B"):
                    return
                g_base = 0
                for bi, batch in enumerate(cfg.batches):
                    bt = len(batch)
                    nch = bt * (c0 + c1)
                    lb = lp.tile([128, nch, 128], dt.float16, tag="lb")
                    nc.scalar.dma_start(lb[:], stage[:, g_base:g_base + nch, :])
                    gb = lb
                    for i, t in enumerate(batch):
                        agg_ps = psA.tile([f_in, 128], dt.float32, tag="psA")
                        slots = list(range(i * c0, (i + 1) * c0)) + list(
                            range(bt * c0 + i * c1, bt * c0 + (i + 1) * c1)
                        )
                        for j, s in enumerate(slots):
                            g = g_base + s
                            V = vp.tile([128, 128], dt.float16, tag="v")
                            nc.vector.tensor_scalar(
                                V[:],
                                iota_sb[:],
                                edstl_sb[:, g:g + 1],
                                enorm_sb[:, g:g + 1],
                                ALU.is_equal,
                                ALU.mult,
                            )
                            nc.tensor.matmul(
                                agg_ps[:],
                                gb[:, s, 0:f_in],
                                V[:],
                                start=(j == 0),
                                stop=(j == len(slots) - 1),
                            )
                        agg_sb = sp.tile([f_in, 128], dt.float16, tag="agg")
                        nc.scalar.copy(agg_sb[:], agg_ps[:])
                        z_ps = psZ.tile([f_out, 128], dt.float32, tag="psZ")
                        nc.tensor.matmul(z_ps[:], w_sb[:], agg_sb[:], start=True, stop=True)
                        zr_sb = sp.tile([f_out, 128], out_dt, tag="zr")
                        nc.scalar.activation(zr_sb[:], z_ps[:], AF.Relu, bias=b_sb[:], scale=1.0)
                        tr_ps = psT.tile([128, f_out], out_dt, tag="psT")
                        nc.tensor.transpose(tr_ps[:], zr_sb[:], id_sb[:])
                        h_sb = sp.tile([128, f_out], out_dt, tag="h")
                        nc.scalar.copy(h_sb[:], tr_ps[:])
                        rows = cfg.last_rows if t == nt - 1 else 128
                        write_out(t, h_sb, rows)
                    g_base += nch

            w1_out_ref = []

            def w1_out(t, h_sb, rows):
                nc.sync.dma_start(cc_in[t * 128:t * 128 + rows, :], h_sb[0:rows, :])

            w1_out_ref.append(w1_out)

            do_layer(xt.ap(), F0, w1_sb, b1_sb, dt.float16, idA_sb, F1, w1_out, 0,
                     stage1)

            nc.gpsimd.collective_compute(
                "AllGather",
                ALU.bypass,
                replica_groups=[list(range(cfg.n_cores))],
                ins=[cc_in.opt()],
                outs=[cc_out.opt()],
            )

            def w2_out(t, h_sb, rows):
                nc.sync.dma_start(outt.ap()[t * 128:t * 128 + rows, :], h_sb[0:rows, :])

            do_layer(cc_out[:, :], F1, w2_sb, b2_sb, dt.float32, idB_sb, F2, w2_out, 2,
                     stage2)

    nc.compile()
    return nc


# ---------------------------------------------------------------------------
# Entry point
# ---------------------------------------------------------------------------

_CACHE = {}


def kernel(x, edge_index, W1, b1, W2, b2):
    x = np.asarray(x)
    cfg, in_maps = prepare(x, edge_index, W1, b1, W2, b2)

    key = (cfg.n, cfg.n_cores, cfg.c0, cfg.c1)
    nc = _CACHE.get(key)
    if nc is None:
        nc = build_program(cfg)
        _CACHE[key] = nc

    from concourse.bass_utils import run_bass_kernel_spmd

    res = run_bass_kernel_spmd(nc, in_maps, core_ids=list(range(cfg.n_cores)))
    out = np.concatenate([r["out"] for r in res.results], axis=0)
    return np.ascontiguousarray(out.astype(np.float32))
